# revision 1
# baseline (speedup 1.0000x reference)
"""BertBiAttention Trainium2 kernel.

Cross-attention between two streams (B=4, S=2048, HID=768, H=12 heads).
Sharding: 8 cores = (stream s in {1,2}) x (batch b in {0..3}). Each core
computes one stream's full output for one batch element:
    h_s[b] = LayerNorm( attend(q_other, k_own, v_own, mask_own) @ wd + bd + x_own )
No collectives needed; the host stacks per-core outputs.

On-chip layouts (per core, all matmuls bf16 with fp32 PSUM accumulation):
  qT, kT  [768, 2048] bf16  (feature-major; head h at partition rows h*64..)
  v       16 x [128, 12, 65] bf16  (per head: [v*exp(mask) | exp(mask)])
  scoresT [krows, q] in PSUM -> exp(s/8) on ACT -> bf16 (sc->exp->ctx
          software-pipelined; dense steps of the previous q-chunk are
          interleaved between heads as PE fill work)
  ctx     lhsT=[v|em] matmuls accumulate [ctx | denom]; denominators of all
          12 heads batched into one DVE reciprocal, broadcast back via a
          DRAM-bounce stride-0 DMA, normalized with one multiply per head
  dense   per-head K=64 matmuls (+bd via K=1 ones matmul) + residual;
          LayerNorm rstd = exp(-0.5*ln(var+eps)) keeps ACT on one table set.
"""

import numpy as np

import concourse.bass as bass
import concourse.mybir as mybir
import concourse.tile as tile
from concourse import bacc, bass_utils
from concourse.masks import make_identity

B, S, HID, H, HD = 4, 2048, 768, 12, 64
FT = HID // 128   # 6 feature tiles
ST = S // 128     # 16 seq tiles
QT = S // 512     # 4 q chunks
NH = 2            # 768-wide outputs split into 2 x 384
NW = 384
EPS = 1e-12

F32 = mybir.dt.float32
F32R = mybir.dt.float32r
BF16 = mybir.dt.bfloat16
AF = mybir.ActivationFunctionType


def _bcast_part(ap, p=128):
    """DRAM row [1, N] -> partition-broadcast AP [p, N] (stride-0 partition)."""
    return bass.AP(tensor=ap.tensor, offset=ap.offset, ap=[[0, p], ap.ap[-1]])


def _setup_act_tables():
    """Point the compiler at an act_info.json whose first set covers both
    exp and ln (natural_log_exp_and_others), so the kernel's Exp and Ln
    activations share one ACT table set instead of reloading (~1.3us) on
    every switch."""
    import json
    import os
    import tempfile
    from pathlib import Path

    if os.environ.get("BASS_ACT_ROOT_JSON_PATH"):
        return
    try:
        from neuronxcc.driver.Job import Job
        from neuronxcc.driver.jobs.support.FindActInfo import findActInfoFile

        src = Path(findActInfoFile(Job.getPackageDir(), "gen3"))
        d = json.loads(src.read_text())
        sets = d["act_func_sets"]
        pref = [s for s in sets if s["name"] == "natural_log_exp_and_others"]
        rest = [s for s in sets if s["name"] != "natural_log_exp_and_others"]
        if not pref:
            return
        d["act_func_sets"] = pref + rest
        dst = Path(tempfile.mkdtemp(prefix="act_tables_"))
        for f in src.parent.iterdir():
            if f.name != src.name and f.is_file():
                os.symlink(f, dst / f.name)
        (dst / src.name).write_text(json.dumps(d))
        os.environ["BASS_ACT_ROOT_JSON_PATH"] = str(dst / src.name)
    except Exception:
        pass  # default tables still work, just slower


def build_nc():
    # _setup_act_tables()  # crashes the exec unit via this compile path
    nc = bacc.Bacc("TRN2", target_bir_lowering=False, debug=False, num_devices=8)

    xq_d = nc.dram_tensor("xq", [S, HID], F32, kind="ExternalInput").ap()
    xkv_d = nc.dram_tensor("xkv", [S, HID], F32, kind="ExternalInput").ap()
    wq_d = nc.dram_tensor("wq", [HID, HID], F32, kind="ExternalInput").ap()
    wk_d = nc.dram_tensor("wk", [HID, HID], F32, kind="ExternalInput").ap()
    wv_d = nc.dram_tensor("wv", [HID, HID], F32, kind="ExternalInput").ap()
    wd_d = nc.dram_tensor("wd", [HID, HID], F32, kind="ExternalInput").ap()
    bq_d = nc.dram_tensor("bq", [1, HID], F32, kind="ExternalInput").ap()
    bk_d = nc.dram_tensor("bk", [1, HID], F32, kind="ExternalInput").ap()
    bv_d = nc.dram_tensor("bv", [1, HID], F32, kind="ExternalInput").ap()
    bd_d = nc.dram_tensor("bd", [1, HID], F32, kind="ExternalInput").ap()
    mask_d = nc.dram_tensor("mask", [S, 1], F32, kind="ExternalInput").ap()
    lng_d = nc.dram_tensor("lng", [1, HID], F32, kind="ExternalInput").ap()
    lnb_d = nc.dram_tensor("lnb", [1, HID], F32, kind="ExternalInput").ap()
    out_d = nc.dram_tensor("out", [S, HID], F32, kind="ExternalOutput").ap()

    with tile.TileContext(nc) as tc:
        with (
            tc.tile_pool(name="consts", bufs=1) as consts,
            tc.tile_pool(name="big", bufs=1) as big,
        ):
            # ---- constants ----
            ident = consts.tile([128, 128], F32)
            make_identity(nc, ident)
            ones_r = consts.tile([1, 128], BF16)
            nc.vector.memset(ones_r, 1.0)
            ones_12 = consts.tile([128, 12], F32)
            nc.vector.memset(ones_12, 1.0)
            eps_t = consts.tile([128, 1], F32)
            nc.vector.memset(eps_t, EPS)

            bqc = consts.tile([128, FT], F32)
            bkc = consts.tile([128, FT], F32)
            for f in range(FT):
                nc.sync.dma_start(
                    out=bqc[:, f : f + 1],
                    in_=bq_d[0:1, f * 128 : (f + 1) * 128].rearrange("a b -> b a"),
                )
                nc.sync.dma_start(
                    out=bkc[:, f : f + 1],
                    in_=bk_d[0:1, f * 128 : (f + 1) * 128].rearrange("a b -> b a"),
                )
            bv_f = consts.tile([1, HID], F32)
            nc.sync.dma_start(out=bv_f, in_=bv_d)
            bd_f = consts.tile([1, HID], F32)
            nc.sync.dma_start(out=bd_f, in_=bd_d)
            bv_row = consts.tile([1, HID], BF16)
            nc.vector.tensor_copy(out=bv_row, in_=bv_f)
            bd_row = consts.tile([1, HID], BF16)
            nc.vector.tensor_copy(out=bd_row, in_=bd_f)

            mask_t = consts.tile([128, ST], F32)
            for t in range(ST):
                nc.sync.dma_start(
                    out=mask_t[:, t : t + 1], in_=mask_d[t * 128 : (t + 1) * 128, :]
                )
            emask = consts.tile([128, ST], F32)
            nc.scalar.activation(out=emask, in_=mask_t, func=AF.Exp)

            # broadcast ln gamma/beta to all 128 partitions (stride-0 DMA)
            g_bc = consts.tile([128, HID], F32)
            b_bc = consts.tile([128, HID], F32)
            nc.sync.dma_start(out=g_bc, in_=_bcast_part(lng_d))
            nc.sync.dma_start(out=b_bc, in_=_bcast_part(lnb_d))

            # ---- persistent activation buffers ----
            qT = [big.tile([128, S], BF16, name=f"qT{f}") for f in range(FT)]
            kT = [big.tile([128, S], BF16, name=f"kT{f}") for f in range(FT)]
            vb = [big.tile([128, H, HD + 1], BF16, name=f"vb{t}") for t in range(ST)]
            # wd stored per-head ([64, 768] at partition base 0) so the dense
            # per-head K=64 matmuls have base-aligned lhsT/rhs
            dw_bf = [big.tile([HD, HID], BF16, name=f"dwbf{h}") for h in range(H)]

            # ---- projections ----
            def project_chunk(x_d, xT_c, ps_tp, xn_pool, chunk):
                """DMA 512 rows of x, transpose into xT_c [128, FT, 512] f32."""
                for ss in range(4):
                    x_nat = xn_pool.tile([128, HID], F32, name="x_nat")
                    st = chunk * 4 + ss
                    nc.sync.dma_start(
                        out=x_nat, in_=x_d[st * 128 : (st + 1) * 128, :]
                    )
                    for f in range(FT):
                        tp_ps = ps_tp.tile([128, 128], F32, name="tp_ps")
                        nc.tensor.transpose(
                            tp_ps, x_nat[:, f * 128 : (f + 1) * 128], ident
                        )
                        nc.vector.tensor_copy(
                            out=xT_c[:, f, ss * 128 : (ss + 1) * 128], in_=tp_ps
                        )

            with (
                tc.tile_pool(name="wkv_pool", bufs=1) as wkv_pool,
                tc.tile_pool(name="xn2", bufs=3) as xn2_pool,
                tc.tile_pool(name="xT2", bufs=2) as xT2_pool,
                tc.tile_pool(name="ps_tp2", bufs=2, space="PSUM") as ps_tp2,
                tc.tile_pool(name="ps_pj2", bufs=2, space="PSUM") as ps_pj2,
                tc.tile_pool(name="ps_v", bufs=2, space="PSUM") as ps_v,
            ):
                wk_b = [
                    wkv_pool.tile([128, HID], BF16, name=f"wk{f}") for f in range(FT)
                ]
                wv_b = [
                    wkv_pool.tile([128, HID], BF16, name=f"wv{f}") for f in range(FT)
                ]
                for f in range(FT):
                    wtmp = xn2_pool.tile([128, HID], F32, name="wtmp2")
                    nc.sync.dma_start(out=wtmp, in_=wk_d[f * 128 : (f + 1) * 128, :])
                    nc.vector.tensor_copy(out=wk_b[f], in_=wtmp)
                    wtmp = xn2_pool.tile([128, HID], F32, name="wtmp2")
                    nc.sync.dma_start(out=wtmp, in_=wv_d[f * 128 : (f + 1) * 128, :])
                    nc.vector.tensor_copy(out=wv_b[f], in_=wtmp)

                for chunk in range(QT):
                    xT_c = xT2_pool.tile([128, FT, 512], BF16, name="xT_kv")
                    project_chunk(xkv_d, xT_c, ps_tp2, xn2_pool, chunk)
                    # kT
                    for fo in range(FT):
                        pj = ps_pj2.tile([128, 512], F32, name="pj2")
                        for kf in range(FT):
                            nc.tensor.matmul(
                                pj,
                                wk_b[kf][:, fo * 128 : (fo + 1) * 128],
                                xT_c[:, kf, :],
                                start=(kf == 0),
                                stop=(kf == FT - 1),
                            )
                        nc.vector.tensor_scalar_add(
                            out=kT[fo][:, chunk * 512 : (chunk + 1) * 512],
                            in0=pj,
                            scalar1=bkc[:, fo : fo + 1],
                        )
                    # v (natural layout, rows scaled by exp(mask), + denom col)
                    for ss in range(4):
                        st = chunk * 4 + ss
                        vp = ps_v.tile([128, NH, 512], F32, name="vp")
                        for nh in range(NH):
                            for kf in range(FT):
                                nc.tensor.matmul(
                                    vp[:, nh, 0:NW],
                                    xT_c[:, kf, ss * 128 : (ss + 1) * 128],
                                    wv_b[kf][:, nh * NW : (nh + 1) * NW],
                                    start=(kf == 0),
                                    stop=False,
                                )
                            nc.tensor.matmul(
                                vp[:, nh, 0:NW],
                                ones_r,
                                bv_row[0:1, nh * NW : (nh + 1) * NW],
                                start=False,
                                stop=True,
                            )
                        emcol = emask[:, st : st + 1]
                        for nh in range(NH):
                            nc.vector.tensor_scalar_mul(
                                out=vb[st][:, nh * 6 : (nh + 1) * 6, 0:HD],
                                in0=vp[:, nh, 0:NW].rearrange(
                                    "p (a d) -> p a d", a=6
                                ),
                                scalar1=emcol,
                            )
                        nc.vector.tensor_scalar_mul(
                            out=vb[st][:, :, HD : HD + 1].rearrange(
                                "p a c -> p (a c)"
                            ),
                            in0=ones_12,
                            scalar1=emcol,
                        )

            # ---- attention + dense + layernorm, per 512-wide q chunk ----
            with (
                tc.tile_pool(name="wq_pool", bufs=1) as wq_pool,
                tc.tile_pool(name="xnq", bufs=2) as xnq_pool,
                tc.tile_pool(name="xTq", bufs=1) as xTq_pool,
                tc.tile_pool(name="ps_tp", bufs=1, space="PSUM") as ps_tp,
                tc.tile_pool(name="ps_pj", bufs=1, space="PSUM") as ps_pj,
                tc.tile_pool(name="ctx_pool", bufs=2) as ctx_pool,
                tc.tile_pool(name="dram_pool", bufs=2, space="DRAM") as dram_pool,
                tc.tile_pool(name="exp_pool", bufs=6) as exp_pool,
                tc.tile_pool(name="rec_pool", bufs=2) as rec_pool,
                tc.tile_pool(name="res_pool", bufs=3) as res_pool,
                tc.tile_pool(name="hpre_pool", bufs=1) as hpre_pool,
                tc.tile_pool(name="st_pool", bufs=4) as st_pool,
                tc.tile_pool(name="ps_sc", bufs=2, space="PSUM") as ps_sc,
                tc.tile_pool(name="ps_ctx", bufs=1, space="PSUM") as ps_ctx,
                tc.tile_pool(name="ps_h", bufs=1, space="PSUM") as ps_h,
            ):
                wq_b = [
                    wq_pool.tile([128, HID], BF16, name=f"wq{f}") for f in range(FT)
                ]
                for f in range(FT):
                    wtmp = xnq_pool.tile([128, HID], F32, name="x_nat")
                    nc.sync.dma_start(out=wtmp, in_=wq_d[f * 128 : (f + 1) * 128, :])
                    nc.vector.tensor_copy(out=wq_b[f], in_=wtmp)
                for h in range(H):
                    wd_t = xnq_pool.tile([HD, HID], F32, name="wd_t")
                    nc.sync.dma_start(out=wd_t, in_=wd_d[h * HD : (h + 1) * HD, :])
                    nc.vector.tensor_copy(out=dw_bf[h], in_=wd_t)

                def q_proj_mm(chunk, xT_c, fo_range):
                    for fo in fo_range:
                        pj = ps_pj.tile([128, 512], F32, name="pj")
                        for kf in range(FT):
                            nc.tensor.matmul(
                                pj,
                                wq_b[kf][:, fo * 128 : (fo + 1) * 128],
                                xT_c[:, kf, :],
                                start=(kf == 0),
                                stop=(kf == FT - 1),
                            )
                        nc.vector.tensor_scalar_add(
                            out=qT[fo][:, chunk * 512 : (chunk + 1) * 512],
                            in0=pj,
                            scalar1=bqc[:, fo : fo + 1],
                        )

                def q_transpose_ss(xT_c, chunk, ss):
                    x_nat = xnq_pool.tile([128, HID], F32, name="x_nat")
                    st = chunk * 4 + ss
                    nc.sync.dma_start(
                        out=x_nat, in_=xq_d[st * 128 : (st + 1) * 128, :]
                    )
                    for f in range(FT):
                        tp_ps = ps_tp.tile([128, 128], F32, name="tp_q")
                        nc.tensor.transpose(
                            tp_ps, x_nat[:, f * 128 : (f + 1) * 128], ident
                        )
                        nc.vector.tensor_copy(
                            out=xT_c[:, f, ss * 128 : (ss + 1) * 128], in_=tp_ps
                        )

                def q_proj_steps(chunk):
                    state = {}

                    def tstep(ss_pair):
                        def run():
                            if "xT" not in state:
                                state["xT"] = xTq_pool.tile(
                                    [128, FT, 512], BF16, name="xT_q"
                                )
                            for ss in ss_pair:
                                q_transpose_ss(state["xT"], chunk, ss)

                        return run

                    def mstep(fo_range):
                        return lambda: q_proj_mm(chunk, state["xT"], fo_range)

                    return [
                        tstep((0, 1)),
                        tstep((2, 3)),
                        mstep(range(0, 2)),
                        mstep(range(2, 4)),
                        mstep(range(4, 6)),
                    ]

                def make_dense_steps(qt, ctx_t):
                    """Dense + residual + LN for chunk qt as 9 deferred steps,
                    emitted between the next chunk's attention heads so the
                    in-order PE has fill work while ACT computes exps."""
                    state = {}

                    def group_step(ss, nh):
                        def run():
                            if "mvq" not in state:
                                state["mvq"] = st_pool.tile(
                                    [128, 4, 2], F32, name="mvq"
                                )
                                state["hp"] = {}
                            st = qt * 4 + ss
                            ssl = slice(ss * 128, (ss + 1) * 128)
                            if ss not in state["hp"]:
                                state["hp"][ss] = hpre_pool.tile(
                                    [128, HID], F32, name=f"hp{ss}"
                                )
                            hp = state["hp"][ss]
                            h_ps = ps_h.tile([128, 512], F32, name="h_ps")
                            for hh in range(H):
                                nc.tensor.matmul(
                                    h_ps[:, 0:NW],
                                    ctx_t[hh][:, ssl],
                                    dw_bf[hh][:, nh * NW : (nh + 1) * NW],
                                    start=(hh == 0),
                                    stop=False,
                                )
                            nc.tensor.matmul(
                                h_ps[:, 0:NW],
                                ones_r,
                                bd_row[0:1, nh * NW : (nh + 1) * NW],
                                start=False,
                                stop=True,
                            )
                            x_res = res_pool.tile([128, NW], F32, name="x_res")
                            nc.sync.dma_start(
                                out=x_res,
                                in_=xkv_d[
                                    st * 128 : (st + 1) * 128,
                                    nh * NW : (nh + 1) * NW,
                                ],
                            )
                            nc.vector.tensor_add(
                                out=hp[:, nh * NW : (nh + 1) * NW],
                                in0=h_ps[:, 0:NW],
                                in1=x_res,
                            )
                            if nh == NH - 1:
                                stats = st_pool.tile([128, 3, 6], F32, name="stats")
                                for sg in range(3):
                                    nc.vector.bn_stats(
                                        out=stats[:, sg, :],
                                        in_=hp[:, sg * 256 : (sg + 1) * 256],
                                    )
                                nc.vector.bn_aggr(
                                    out=state["mvq"][:, ss, :], in_=stats
                                )

                        return run

                    def tail():
                        mvq = state["mvq"]
                        # rstd = exp(-0.5*ln(var+eps)), batched over the 4
                        # subtiles (2 ACT table switches per chunk)
                        lnv = st_pool.tile([128, 4], F32, name="lnv")
                        nc.scalar.activation(
                            out=lnv, in_=mvq[:, :, 1], func=AF.Ln,
                            bias=eps_t, scale=1.0,
                        )
                        rstd4 = st_pool.tile([128, 4], F32, name="rstd4")
                        nc.scalar.activation(
                            out=rstd4, in_=lnv, func=AF.Exp, scale=-0.5
                        )
                        for ss in range(4):
                            st = qt * 4 + ss
                            hp = state["hp"][ss]
                            hn = hpre_pool.tile([128, HID], F32, name="hn")
                            nc.vector.tensor_scalar(
                                out=hn,
                                in0=hp,
                                scalar1=mvq[:, ss, 0:1],
                                scalar2=rstd4[:, ss : ss + 1],
                                op0=mybir.AluOpType.subtract,
                                op1=mybir.AluOpType.mult,
                            )
                            nc.vector.tensor_mul(hn, hn, g_bc)
                            nc.vector.tensor_add(hn, hn, b_bc)
                            nc.sync.dma_start(
                                out=out_d[st * 128 : (st + 1) * 128, :], in_=hn
                            )

                    return [group_step(ss, nh) for ss in range(4) for nh in range(NH)] + [tail]

                # chunk 0's qT is needed immediately; emit it directly
                xT0 = xTq_pool.tile([128, FT, 512], BF16, name="xT_q")
                for ss in range(4):
                    q_transpose_ss(xT0, 0, ss)
                q_proj_mm(0, xT0, range(FT))

                pending = []

                def pop_fill():
                    if pending:
                        pending.pop(0)()

                def emit_head(qt, h, ctx_t, den_all):
                    qsl = slice(qt * 512, (qt + 1) * 512)
                    ft, po = h // 2, (h % 2) * 64
                    ctx_ps = ps_ctx.tile([HD + 1, 512], F32, name="ctx_ps")
                    # software-pipelined: scores(g) before ctx(g-1) so the
                    # in-order PE streams scores while ACT computes exp(g-1)
                    exps = []
                    for g in range(8):
                        sc_ps = ps_sc.tile([128, 2, 512], F32, name="sc_ps")
                        for j in range(2):
                            kc = g * 2 + j
                            nc.tensor.matmul(
                                sc_ps[:, j, :],
                                kT[ft][po : po + HD, kc * 128 : (kc + 1) * 128],
                                qT[ft][po : po + HD, qsl],
                                start=True,
                                stop=True,
                            )
                        exp_g = exp_pool.tile([128, 2, 512], BF16, name="exp_g")
                        nc.scalar.activation(
                            out=exp_g, in_=sc_ps, func=AF.Exp, scale=0.125
                        )
                        exps.append(exp_g)
                        if g == 4:
                            pop_fill()  # fill PE while ACT works
                        if g > 0:
                            for j in range(2):
                                kc = (g - 1) * 2 + j
                                nc.tensor.matmul(
                                    ctx_ps,
                                    vb[kc][:, h, :],
                                    exps[g - 1][:, j, :],
                                    start=(g == 1 and j == 0),
                                    stop=False,
                                )
                    for j in range(2):
                        kc = 7 * 2 + j
                        nc.tensor.matmul(
                            ctx_ps,
                            vb[kc][:, h, :],
                            exps[7][:, j, :],
                            start=False,
                            stop=(j == 1),
                        )
                    # evict unnormalized ctx + denominator; normalization is
                    # batched per chunk, off the head loop
                    dtmp = rec_pool.tile([HD + 1, 512], F32, name="dtmp")
                    nc.vector.tensor_copy(
                        out=dtmp[HD : HD + 1, :], in_=ctx_ps[HD : HD + 1, :]
                    )
                    nc.sync.dma_start(
                        out=den_all[h : h + 1, :], in_=dtmp[HD : HD + 1, :]
                    )
                    nc.vector.tensor_copy(out=ctx_t[h], in_=ctx_ps[0:HD, :])

                def emit_norm(ctx_t, den_all):
                    # one iterative-divide pass for all 12 heads' denoms;
                    # partition-broadcast via DRAM bounce + stride-0 DMA
                    rec_all = rec_pool.tile([H, 512], F32, name="rec_all")
                    nc.vector.reciprocal(rec_all, den_all)
                    rec_d = dram_pool.tile([H, 512], F32, name="rec_d")
                    nc.sync.dma_start(out=rec_d, in_=rec_all)
                    for h in range(H):
                        bc_sb = rec_pool.tile([HD, 512], F32, name="bc_sb")
                        nc.sync.dma_start(
                            out=bc_sb,
                            in_=rec_d[h : h + 1, :].to_broadcast((HD, 512)),
                        )
                        nc.vector.tensor_mul(
                            out=ctx_t[h], in0=ctx_t[h], in1=bc_sb
                        )
                        if h % 2 == 0:
                            pop_fill()

                for qt in range(QT):
                    if qt + 1 < QT:
                        pending.extend(q_proj_steps(qt + 1))
                    ctx_t = [
                        ctx_pool.tile([HD, 512], BF16, name=f"ctx{h}")
                        for h in range(H)
                    ]
                    den_all = rec_pool.tile([H, 512], F32, name="den_all")
                    for h in range(H):
                        emit_head(qt, h, ctx_t, den_all)
                    emit_norm(ctx_t, den_all)
                    pending.extend(make_dense_steps(qt, ctx_t))
                for step in pending:
                    step()

    nc.compile()
    return nc


_NC = None


def _get_nc():
    global _NC
    if _NC is None:
        _NC = build_nc()
    return _NC


def _prepare(
    input_tensor1, attention_mask1, input_tensor2, attention_mask2,
    q1_w, q1_b, k1_w, k1_b, v1_w, v1_b,
    q2_w, q2_b, k2_w, k2_b, v2_w, v2_b,
    d1_w, d1_b, d2_w, d2_b, ln1_g, ln1_b, ln2_g, ln2_b,
):
    f = lambda a: np.ascontiguousarray(np.asarray(a), dtype=np.float32)
    x1, x2 = f(input_tensor1), f(input_tensor2)
    m1 = f(attention_mask1).reshape(B, S, 1)
    m2 = f(attention_mask2).reshape(B, S, 1)
    row = lambda a: f(a).reshape(1, HID)

    in_maps = []
    for b in range(B):
        # stream1: ctx1 = attend(q2, k1, v1, mask1); out h1[b]
        in_maps.append({
            "xq": x2[b], "xkv": x1[b],
            "wq": f(q2_w), "wk": f(k1_w), "wv": f(v1_w), "wd": f(d1_w),
            "bq": row(q2_b), "bk": row(k1_b), "bv": row(v1_b), "bd": row(d1_b),
            "mask": m1[b], "lng": row(ln1_g), "lnb": row(ln1_b),
        })
    for b in range(B):
        # stream2: ctx2 = attend(q1, k2, v2, mask2); out h2[b]
        in_maps.append({
            "xq": x1[b], "xkv": x2[b],
            "wq": f(q1_w), "wk": f(k2_w), "wv": f(v2_w), "wd": f(d2_w),
            "bq": row(q1_b), "bk": row(k2_b), "bv": row(v2_b), "bd": row(d2_b),
            "mask": m2[b], "lng": row(ln2_g), "lnb": row(ln2_b),
        })

    return in_maps


def _run(in_maps, **kwargs):
    nc = _get_nc()
    res = bass_utils.run_bass_kernel_spmd(
        nc, in_maps, core_ids=list(range(8)), **kwargs
    )
    h1 = np.stack([res.results[b]["out"] for b in range(B)])
    h2 = np.stack([res.results[B + b]["out"] for b in range(B)])
    return (h1, h2), res


def kernel(**inputs):
    (h1, h2), _ = _run(_prepare(**inputs))
    return h1, h2



# revision 5
# speedup vs baseline: 1.0771x; 1.0771x over previous
"""BertBiAttention Trainium2 kernel (v2).

Cross-attention between two streams (B=4, S=2048, HID=768, H=12 heads).
Sharding: 8 cores = (stream s in {1,2}) x (batch b in {0..3}). Each core
computes one stream's full output for one batch element:
    h_s[b] = LayerNorm( attend(q_other, k_own, v_own, mask_own) @ wd + bd + x_own )
No collectives needed; the host stacks per-core outputs.

v2 structure (per core):
  - scores: bf16 row-tiled CONCURRENT matmul pairs -- heads (2f, 2f+1) live at
    partition rows 0:64 / 64:128 of qT/kT[f]; tile_position (0,0)/(64,0) runs
    both K=64 matmuls simultaneously on the PE array (~2x score throughput).
  - softmax exp on ACT writes fp8e4 directly; ctx matmuls are fp8 DoubleRow
    (two key-tiles per pass, ~1.8x) with lhsT = packed v [128, 2, 12, 80]
    ([v*exp(mask) | exp(mask)] per head, denom row included, M=65).
  - ctx evicted into ctx2 [128, 6, 512] (heads packed along partitions via
    DVE partition-base-shifted copies) so dense runs K=128 matmuls (2x);
    denominators batched: reciprocal_approx_fast + bf16 DRAM-bounce
    broadcast + one in-place 2x-mode multiply per head.
  - v-projection deferred out of the startup phase: emitted inline between
    pair-0 score steps so it runs while ACT computes exps.
  - dense + residual + LayerNorm (rstd = exp(-0.5*ln(var+eps)) keeps ACT on
    one table set) deferred as fill work into the next chunk's attention.
"""

import numpy as np

import concourse.bass as bass
import concourse.mybir as mybir
import concourse.tile as tile
from concourse import bacc, bass_utils
from concourse.masks import make_identity

B, S, HID, H, HD = 4, 2048, 768, 12, 64
FT = HID // 128   # 6 feature tiles
ST = S // 128     # 16 seq tiles
QT = S // 512     # 4 q chunks
NH = 2            # 768-wide outputs split into 2 x 384
NW = 384
VW = 80           # padded per-head width in packed v (64 ctx + 1 denom + pad)
EPS = 1e-12

F32 = mybir.dt.float32
BF16 = mybir.dt.bfloat16
FP8 = mybir.dt.float8e4
AF = mybir.ActivationFunctionType
DR = mybir.MatmulPerfMode.DoubleRow


def _bcast_part(ap, p=128):
    """DRAM row [1, N] -> partition-broadcast AP [p, N] (stride-0 partition)."""
    return bass.AP(tensor=ap.tensor, offset=ap.offset, ap=[[0, p], ap.ap[-1]])


def build_nc():
    nc = bacc.Bacc("TRN2", target_bir_lowering=False, debug=False, num_devices=8)

    xq_d = nc.dram_tensor("xq", [S, HID], F32, kind="ExternalInput").ap()
    xkv_d = nc.dram_tensor("xkv", [S, HID], F32, kind="ExternalInput").ap()
    wq_d = nc.dram_tensor("wq", [HID, HID], F32, kind="ExternalInput").ap()
    wk_d = nc.dram_tensor("wk", [HID, HID], F32, kind="ExternalInput").ap()
    wv_d = nc.dram_tensor("wv", [HID, HID], F32, kind="ExternalInput").ap()
    wd_d = nc.dram_tensor("wd", [HID, HID], F32, kind="ExternalInput").ap()
    bq_d = nc.dram_tensor("bq", [1, HID], F32, kind="ExternalInput").ap()
    bk_d = nc.dram_tensor("bk", [1, HID], F32, kind="ExternalInput").ap()
    bv_d = nc.dram_tensor("bv", [1, HID], F32, kind="ExternalInput").ap()
    bd_d = nc.dram_tensor("bd", [1, HID], F32, kind="ExternalInput").ap()
    mask_d = nc.dram_tensor("mask", [S, 1], F32, kind="ExternalInput").ap()
    lng_d = nc.dram_tensor("lng", [1, HID], F32, kind="ExternalInput").ap()
    lnb_d = nc.dram_tensor("lnb", [1, HID], F32, kind="ExternalInput").ap()
    out_d = nc.dram_tensor("out", [S, HID], F32, kind="ExternalOutput").ap()

    with tile.TileContext(nc) as tc:
        with (
            tc.tile_pool(name="consts", bufs=1) as consts,
            tc.tile_pool(name="big", bufs=1) as big,
        ):
            # ---- constants ----
            ident = consts.tile([128, 128], F32)
            make_identity(nc, ident)
            ones_r = consts.tile([1, 128], BF16)
            nc.vector.memset(ones_r, 1.0)
            ones_12 = consts.tile([128, 12], F32)
            nc.vector.memset(ones_12, 1.0)
            eps_t = consts.tile([128, 1], F32)
            nc.vector.memset(eps_t, EPS)

            bqc = consts.tile([128, FT], F32)
            bkc = consts.tile([128, FT], F32)
            for f in range(FT):
                nc.sync.dma_start(
                    out=bqc[:, f : f + 1],
                    in_=bq_d[0:1, f * 128 : (f + 1) * 128].rearrange("a b -> b a"),
                )
                nc.sync.dma_start(
                    out=bkc[:, f : f + 1],
                    in_=bk_d[0:1, f * 128 : (f + 1) * 128].rearrange("a b -> b a"),
                )
            bv_f = consts.tile([1, HID], F32)
            nc.sync.dma_start(out=bv_f, in_=bv_d)
            bd_f = consts.tile([1, HID], F32)
            nc.sync.dma_start(out=bd_f, in_=bd_d)
            bv_row = consts.tile([1, HID], BF16)
            nc.vector.tensor_copy(out=bv_row, in_=bv_f)
            bd_row = consts.tile([1, HID], BF16)
            nc.vector.tensor_copy(out=bd_row, in_=bd_f)

            mask_t = consts.tile([128, ST], F32)
            for t in range(ST):
                nc.sync.dma_start(
                    out=mask_t[:, t : t + 1], in_=mask_d[t * 128 : (t + 1) * 128, :]
                )
            emask = consts.tile([128, ST], F32)
            nc.scalar.activation(out=emask, in_=mask_t, func=AF.Exp)

            # broadcast ln gamma/beta to all 128 partitions (stride-0 DMA)
            g_bc = consts.tile([128, HID], F32)
            b_bc = consts.tile([128, HID], F32)
            nc.sync.dma_start(out=g_bc, in_=_bcast_part(lng_d))
            nc.sync.dma_start(out=b_bc, in_=_bcast_part(lnb_d))

            # ---- persistent activation buffers ----
            qT = [big.tile([128, S], BF16, name=f"qT{f}") for f in range(FT)]
            kT = [big.tile([128, S], BF16, name=f"kT{f}") for f in range(FT)]
            # packed v, fp8: [keys, ktile-pair-half, head, 64 ctx + denom]
            vb_dr = [
                big.tile([128, 2, H, VW], FP8, name=f"vbdr{u}") for u in range(8)
            ]
            # wd natural rows, bf16 (dense is K=128 over packed ctx2)
            dw6 = [big.tile([128, HID], BF16, name=f"dw6{f}") for f in range(FT)]
            # transposed x_kv chunks stay alive for the deferred v projection
            xTkv = [
                big.tile([128, FT, 512], BF16, name=f"xTkv{c}") for c in range(QT)
            ]
            wv_b = [big.tile([128, HID], BF16, name=f"wv{f}") for f in range(FT)]
            wq_b = [big.tile([128, HID], BF16, name=f"wq{f}") for f in range(FT)]

            # ---- startup: transpose x_kv, project kT (v deferred) ----
            with (
                tc.tile_pool(name="wk_pool", bufs=1) as wk_pool,
                tc.tile_pool(name="xn2", bufs=3) as xn2_pool,
                tc.tile_pool(name="ps_tp2", bufs=2, space="PSUM") as ps_tp2,
                tc.tile_pool(name="ps_pj2", bufs=2, space="PSUM") as ps_pj2,
            ):
                wk_b = [
                    wk_pool.tile([128, HID], BF16, name=f"wk{f}") for f in range(FT)
                ]
                for f in range(FT):
                    wtmp = xn2_pool.tile([128, HID], F32, name="wtmp2")
                    nc.sync.dma_start(out=wtmp, in_=wk_d[f * 128 : (f + 1) * 128, :])
                    nc.vector.tensor_copy(out=wk_b[f], in_=wtmp)
                    wtmp = xn2_pool.tile([128, HID], F32, name="wtmp2")
                    nc.sync.dma_start(out=wtmp, in_=wv_d[f * 128 : (f + 1) * 128, :])
                    nc.vector.tensor_copy(out=wv_b[f], in_=wtmp)

                for chunk in range(QT):
                    xT_c = xTkv[chunk]
                    for ss in range(4):
                        x_nat = xn2_pool.tile([128, HID], F32, name="x_nat")
                        st = chunk * 4 + ss
                        nc.sync.dma_start(
                            out=x_nat, in_=xkv_d[st * 128 : (st + 1) * 128, :]
                        )
                        for f in range(FT):
                            tp_ps = ps_tp2.tile([128, 128], F32, name="tp_ps")
                            nc.tensor.transpose(
                                tp_ps, x_nat[:, f * 128 : (f + 1) * 128], ident
                            )
                            nc.vector.tensor_copy(
                                out=xT_c[:, f, ss * 128 : (ss + 1) * 128], in_=tp_ps
                            )
                    for fo in range(FT):
                        pj = ps_pj2.tile([128, 512], F32, name="pj2")
                        for kf in range(FT):
                            nc.tensor.matmul(
                                pj,
                                wk_b[kf][:, fo * 128 : (fo + 1) * 128],
                                xT_c[:, kf, :],
                                start=(kf == 0),
                                stop=(kf == FT - 1),
                            )
                        nc.vector.tensor_scalar_add(
                            out=kT[fo][:, chunk * 512 : (chunk + 1) * 512],
                            in0=pj,
                            scalar1=bkc[:, fo : fo + 1],
                        )

            # ---- attention + dense + layernorm ----
            with (
                tc.tile_pool(name="xnq", bufs=2) as xnq_pool,
                tc.tile_pool(name="xTq", bufs=1) as xTq_pool,
                tc.tile_pool(name="ctx2_pool", bufs=2) as ctx2_pool,
                tc.tile_pool(name="exp_pool", bufs=3) as exp_pool,
                tc.tile_pool(name="dram_pool", bufs=2, space="DRAM") as dram_pool,
                tc.tile_pool(name="rec_pool", bufs=2) as rec_pool,
                tc.tile_pool(name="res_pool", bufs=3) as res_pool,
                tc.tile_pool(name="hpre_pool", bufs=1) as hpre_pool,
                tc.tile_pool(name="st_pool", bufs=4) as st_pool,
                tc.tile_pool(name="ps_sc", bufs=2, space="PSUM") as ps_sc,
                tc.tile_pool(name="ps_ctx", bufs=1, space="PSUM") as ps_ctx,
                tc.tile_pool(name="ps_aux", bufs=2, space="PSUM") as ps_aux,
            ):
                for f in range(FT):
                    wtmp = xnq_pool.tile([128, HID], F32, name="x_nat")
                    nc.sync.dma_start(out=wtmp, in_=wq_d[f * 128 : (f + 1) * 128, :])
                    nc.vector.tensor_copy(out=wq_b[f], in_=wtmp)
                for f in range(FT):
                    wtmp = xnq_pool.tile([128, HID], F32, name="x_nat")
                    nc.sync.dma_start(out=wtmp, in_=wd_d[f * 128 : (f + 1) * 128, :])
                    nc.vector.tensor_copy(out=dw6[f], in_=wtmp)

                # --- deferred v projection: one 128-row step, both halves ---
                def v_proj_step(st):
                    u, half = st // 2, st % 2
                    chunk, ss = st // 4, st % 4
                    xT_c = xTkv[chunk]
                    emcol = emask[:, st : st + 1]
                    for nh in range(NH):
                        vp = ps_aux.tile([128, 512], F32, name="aux")
                        for kf in range(FT):
                            nc.tensor.matmul(
                                vp[:, 0:NW],
                                xT_c[:, kf, ss * 128 : (ss + 1) * 128],
                                wv_b[kf][:, nh * NW : (nh + 1) * NW],
                                start=(kf == 0),
                                stop=False,
                            )
                        nc.tensor.matmul(
                            vp[:, 0:NW],
                            ones_r,
                            bv_row[0:1, nh * NW : (nh + 1) * NW],
                            start=False,
                            stop=True,
                        )
                        nc.vector.tensor_scalar_mul(
                            out=vb_dr[u][:, half, nh * 6 : (nh + 1) * 6, 0:HD],
                            in0=vp[:, 0:NW].rearrange("p (a d) -> p a d", a=6),
                            scalar1=emcol,
                        )
                    nc.vector.tensor_scalar_mul(
                        out=vb_dr[u][:, half, :, HD : HD + 1].rearrange(
                            "p a c -> p (a c)"
                        ),
                        in0=ones_12,
                        scalar1=emcol,
                    )

                # --- q projection (fill work) ---
                def q_proj_mm(chunk, xT_c, fo_range):
                    for fo in fo_range:
                        pj = ps_aux.tile([128, 512], F32, name="aux")
                        for kf in range(FT):
                            nc.tensor.matmul(
                                pj,
                                wq_b[kf][:, fo * 128 : (fo + 1) * 128],
                                xT_c[:, kf, :],
                                start=(kf == 0),
                                stop=(kf == FT - 1),
                            )
                        nc.vector.tensor_scalar_add(
                            out=qT[fo][:, chunk * 512 : (chunk + 1) * 512],
                            in0=pj,
                            scalar1=bqc[:, fo : fo + 1],
                        )

                def q_transpose_ss(xT_c, chunk, ss):
                    x_nat = xnq_pool.tile([128, HID], F32, name="x_nat")
                    st = chunk * 4 + ss
                    nc.sync.dma_start(
                        out=x_nat, in_=xq_d[st * 128 : (st + 1) * 128, :]
                    )
                    for f in range(FT):
                        tp_ps = ps_aux.tile([128, 512], F32, name="aux")
                        nc.tensor.transpose(
                            tp_ps[:, 0:128], x_nat[:, f * 128 : (f + 1) * 128], ident
                        )
                        nc.vector.tensor_copy(
                            out=xT_c[:, f, ss * 128 : (ss + 1) * 128],
                            in_=tp_ps[:, 0:128],
                        )

                def q_proj_steps(chunk):
                    state = {}

                    def tstep(ss_pair):
                        def run():
                            if "xT" not in state:
                                state["xT"] = xTq_pool.tile(
                                    [128, FT, 512], BF16, name="xT_q"
                                )
                            for ss in ss_pair:
                                q_transpose_ss(state["xT"], chunk, ss)

                        return run

                    def mstep(fo_range):
                        return lambda: q_proj_mm(chunk, state["xT"], fo_range)

                    return [
                        tstep((0, 1)),
                        tstep((2, 3)),
                        mstep(range(0, 2)),
                        mstep(range(2, 4)),
                        mstep(range(4, 6)),
                    ]

                # --- dense + residual + LN for chunk qt (fill work) ---
                def make_dense_steps(qt, ctx2t):
                    state = {}

                    def group_step(ss, nh):
                        def run():
                            if "mvq" not in state:
                                state["mvq"] = st_pool.tile(
                                    [128, 4, 2], F32, name="mvq"
                                )
                                state["hp"] = {}
                            st = qt * 4 + ss
                            ssl = slice(ss * 128, (ss + 1) * 128)
                            if ss not in state["hp"]:
                                state["hp"][ss] = hpre_pool.tile(
                                    [128, HID], F32, name=f"hp{ss}"
                                )
                            hp = state["hp"][ss]
                            h_ps = ps_aux.tile([128, 512], F32, name="aux")
                            for f in range(FT):
                                nc.tensor.matmul(
                                    h_ps[:, 0:NW],
                                    ctx2t[:, f, ssl],
                                    dw6[f][:, nh * NW : (nh + 1) * NW],
                                    start=(f == 0),
                                    stop=False,
                                )
                            nc.tensor.matmul(
                                h_ps[:, 0:NW],
                                ones_r,
                                bd_row[0:1, nh * NW : (nh + 1) * NW],
                                start=False,
                                stop=True,
                            )
                            x_res = res_pool.tile([128, NW], F32, name="x_res")
                            nc.sync.dma_start(
                                out=x_res,
                                in_=xkv_d[
                                    st * 128 : (st + 1) * 128,
                                    nh * NW : (nh + 1) * NW,
                                ],
                            )
                            nc.vector.tensor_add(
                                out=hp[:, nh * NW : (nh + 1) * NW],
                                in0=h_ps[:, 0:NW],
                                in1=x_res,
                            )
                            if nh == NH - 1:
                                stats = st_pool.tile([128, 3, 6], F32, name="stats")
                                for sg in range(3):
                                    nc.vector.bn_stats(
                                        out=stats[:, sg, :],
                                        in_=hp[:, sg * 256 : (sg + 1) * 256],
                                    )
                                nc.vector.bn_aggr(
                                    out=state["mvq"][:, ss, :], in_=stats
                                )

                        return run

                    def tail():
                        mvq = state["mvq"]
                        lnv = st_pool.tile([128, 4], F32, name="lnv")
                        nc.scalar.activation(
                            out=lnv, in_=mvq[:, :, 1], func=AF.Ln,
                            bias=eps_t, scale=1.0,
                        )
                        rstd4 = st_pool.tile([128, 4], F32, name="rstd4")
                        nc.scalar.activation(
                            out=rstd4, in_=lnv, func=AF.Exp, scale=-0.5
                        )
                        for ss in range(4):
                            st = qt * 4 + ss
                            hp = state["hp"][ss]
                            hn = hpre_pool.tile([128, HID], F32, name="hn")
                            nc.vector.tensor_scalar(
                                out=hn,
                                in0=hp,
                                scalar1=mvq[:, ss, 0:1],
                                scalar2=rstd4[:, ss : ss + 1],
                                op0=mybir.AluOpType.subtract,
                                op1=mybir.AluOpType.mult,
                            )
                            nc.vector.tensor_mul(hn, hn, g_bc)
                            nc.vector.tensor_add(hn, hn, b_bc)
                            nc.sync.dma_start(
                                out=out_d[st * 128 : (st + 1) * 128, :], in_=hn
                            )

                    return [
                        group_step(ss, nh) for ss in range(4) for nh in range(NH)
                    ] + [tail]

                # chunk 0's qT is needed immediately; emit it directly
                xT0 = xTq_pool.tile([128, FT, 512], BF16, name="xT_q")
                for ss in range(4):
                    q_transpose_ss(xT0, 0, ss)
                q_proj_mm(0, xT0, range(FT))

                pending = []

                def pop_fill():
                    if pending:
                        pending.pop(0)()

                def emit_pair(qt, hp, ctx2t, den_all, v_inline):
                    """Attention for head pair (2hp, 2hp+1), q chunk qt."""
                    qsl = slice(qt * 512, (qt + 1) * 512)
                    ctxA = ps_ctx.tile([HD + 1, 512], F32, name="ctxA")
                    ctxB = ps_ctx.tile([HD + 1, 512], F32, name="ctxB")
                    exps_u = {}

                    def emit_ctx(u):
                        rv = exps_u[u].rearrange(
                            "p (k two) n -> p two k n", two=2
                        )
                        nc.tensor.matmul(
                            ctxA,
                            vb_dr[u][:, :, 2 * hp, 0 : HD + 1],
                            rv[:, 0],
                            start=(u == 0),
                            stop=(u == 7),
                            perf_mode=DR,
                        )
                        nc.tensor.matmul(
                            ctxB,
                            vb_dr[u][:, :, 2 * hp + 1, 0 : HD + 1],
                            rv[:, 1],
                            start=(u == 0),
                            stop=(u == 7),
                            perf_mode=DR,
                        )

                    for kc in range(ST):
                        u, half = kc // 2, kc % 2
                        sc = ps_sc.tile([128, 2, 512], F32, name="sc")
                        nc.tensor.matmul(
                            sc[:, 0, :],
                            kT[hp][0:HD, kc * 128 : (kc + 1) * 128],
                            qT[hp][0:HD, qsl],
                            start=True, stop=True, tile_position=(0, 0),
                        )
                        nc.tensor.matmul(
                            sc[:, 1, :],
                            kT[hp][HD:128, kc * 128 : (kc + 1) * 128],
                            qT[hp][HD:128, qsl],
                            start=True, stop=True, tile_position=(64, 0),
                        )
                        if v_inline:
                            v_proj_step(kc)
                        if half == 0:
                            exps_u[u] = exp_pool.tile(
                                [128, 4, 512], FP8, name="exps"
                            )
                        nc.scalar.activation(
                            out=exps_u[u][:, 2 * half : 2 * half + 2, :],
                            in_=sc, func=AF.Exp, scale=0.125,
                        )
                        if kc >= 3 and half == 1:
                            emit_ctx((kc - 3) // 2)
                            if not v_inline:
                                pop_fill()
                    emit_ctx(7)

                    # evict: ctx rows into packed ctx2 (partition-shift is
                    # 64 = legal); denom rows via same-partition copy + DMA
                    # (DVE partition shifts must be multiples of 32)
                    dtmp = rec_pool.tile([HD + 1, 1024], F32, name="dtmp")
                    nc.vector.tensor_copy(
                        out=ctx2t[0:HD, hp, :], in_=ctxA[0:HD, :]
                    )
                    nc.vector.tensor_copy(
                        out=dtmp[HD : HD + 1, 0:512], in_=ctxA[HD : HD + 1, :]
                    )
                    nc.vector.tensor_copy(
                        out=ctx2t[HD:128, hp, :], in_=ctxB[0:HD, :]
                    )
                    nc.vector.tensor_copy(
                        out=dtmp[HD : HD + 1, 512:1024], in_=ctxB[HD : HD + 1, :]
                    )
                    nc.sync.dma_start(
                        out=den_all[2 * hp : 2 * hp + 2, :],
                        in_=dtmp[HD : HD + 1, :],
                    )
                    if not v_inline:
                        pop_fill()
                        pop_fill()

                def emit_norm(ctx2t, den_all):
                    # batched reciprocal; bf16 partition-broadcast via DRAM
                    # bounce + stride-0 DMA; one in-place 2x multiply per head
                    rec_all = rec_pool.tile([H, 512], F32, name="rec_all")
                    nc.vector.reciprocal_approx_fast(out=rec_all, in_=den_all)
                    rec_bf = rec_pool.tile([H, 512], BF16, name="rec_bf")
                    nc.vector.tensor_copy(out=rec_bf, in_=rec_all)
                    rec_d = dram_pool.tile([H, 512], BF16, name="rec_d")
                    nc.sync.dma_start(out=rec_d, in_=rec_bf)
                    for f in range(FT):
                        # rec rows for heads (2f, 2f+1) stacked on partitions
                        # 0:64 / 64:128 -> one full-width multiply per f-tile
                        bc_sb = rec_pool.tile([128, 512], BF16, name="bc_sb")
                        nc.sync.dma_start(
                            out=bc_sb[0:HD, :],
                            in_=rec_d[2 * f : 2 * f + 1, :].to_broadcast((HD, 512)),
                        )
                        nc.sync.dma_start(
                            out=bc_sb[HD:128, :],
                            in_=rec_d[2 * f + 1 : 2 * f + 2, :].to_broadcast(
                                (HD, 512)
                            ),
                        )
                        nc.vector.tensor_mul(
                            out=ctx2t[:, f, :],
                            in0=ctx2t[:, f, :],
                            in1=bc_sb,
                        )
                        pop_fill()

                for qt in range(QT):
                    if qt + 1 < QT:
                        pending.extend(q_proj_steps(qt + 1))
                    ctx2t = ctx2_pool.tile([128, FT, 512], BF16, name="ctx2")
                    den_all = rec_pool.tile([H, 512], F32, name="den_all")
                    for hp in range(FT):
                        emit_pair(qt, hp, ctx2t, den_all, qt == 0 and hp == 0)
                    emit_norm(ctx2t, den_all)
                    pending.extend(make_dense_steps(qt, ctx2t))
                for step in pending:
                    step()

    nc.compile()
    return nc


_NC = None


def _get_nc():
    global _NC
    if _NC is None:
        _NC = build_nc()
    return _NC


def _prepare(
    input_tensor1, attention_mask1, input_tensor2, attention_mask2,
    q1_w, q1_b, k1_w, k1_b, v1_w, v1_b,
    q2_w, q2_b, k2_w, k2_b, v2_w, v2_b,
    d1_w, d1_b, d2_w, d2_b, ln1_g, ln1_b, ln2_g, ln2_b,
):
    f = lambda a: np.ascontiguousarray(np.asarray(a), dtype=np.float32)
    x1, x2 = f(input_tensor1), f(input_tensor2)
    m1 = f(attention_mask1).reshape(B, S, 1)
    m2 = f(attention_mask2).reshape(B, S, 1)
    row = lambda a: f(a).reshape(1, HID)

    in_maps = []
    for b in range(B):
        # stream1: ctx1 = attend(q2, k1, v1, mask1); out h1[b]
        in_maps.append({
            "xq": x2[b], "xkv": x1[b],
            "wq": f(q2_w), "wk": f(k1_w), "wv": f(v1_w), "wd": f(d1_w),
            "bq": row(q2_b), "bk": row(k1_b), "bv": row(v1_b), "bd": row(d1_b),
            "mask": m1[b], "lng": row(ln1_g), "lnb": row(ln1_b),
        })
    for b in range(B):
        # stream2: ctx2 = attend(q1, k2, v2, mask2); out h2[b]
        in_maps.append({
            "xq": x1[b], "xkv": x2[b],
            "wq": f(q1_w), "wk": f(k2_w), "wv": f(v2_w), "wd": f(d2_w),
            "bq": row(q1_b), "bk": row(k2_b), "bv": row(v2_b), "bd": row(d2_b),
            "mask": m2[b], "lng": row(ln2_g), "lnb": row(ln2_b),
        })

    return in_maps


def _run(in_maps, **kwargs):
    nc = _get_nc()
    res = bass_utils.run_bass_kernel_spmd(
        nc, in_maps, core_ids=list(range(8)), **kwargs
    )
    h1 = np.stack([res.results[b]["out"] for b in range(B)])
    h2 = np.stack([res.results[B + b]["out"] for b in range(B)])
    return (h1, h2), res


def kernel(**inputs):
    (h1, h2), _ = _run(_prepare(**inputs))
    return h1, h2


# revision 12
# speedup vs baseline: 1.1655x; 1.0820x over previous
"""BertBiAttention Trainium2 kernel (v2).

Cross-attention between two streams (B=4, S=2048, HID=768, H=12 heads).
Sharding: 8 cores = (stream s in {1,2}) x (batch b in {0..3}). Each core
computes one stream's full output for one batch element:
    h_s[b] = LayerNorm( attend(q_other, k_own, v_own, mask_own) @ wd + bd + x_own )
No collectives needed; the host stacks per-core outputs.

v2 structure (per core):
  - scores: bf16 row-tiled CONCURRENT matmul pairs -- heads (2f, 2f+1) live at
    partition rows 0:64 / 64:128 of qT/kT[f]; tile_position (0,0)/(64,0) runs
    both K=64 matmuls simultaneously on the PE array (~2x score throughput).
  - softmax exp on ACT writes fp8e4 directly; ctx matmuls are fp8 DoubleRow
    (two key-tiles per pass, ~1.8x) with lhsT = packed v [128, 2, 12, 80]
    ([v*exp(mask) | exp(mask)] per head, denom row included, M=65).
  - ctx evicted into ctx2 [128, 6, 512] (heads packed along partitions via
    DVE partition-base-shifted copies) so dense runs K=128 matmuls (2x);
    denominators batched: reciprocal_approx_fast + bf16 DRAM-bounce
    broadcast + one in-place 2x-mode multiply per head.
  - v-projection deferred out of the startup phase: emitted inline between
    pair-0 score steps so it runs while ACT computes exps.
  - dense + residual + LayerNorm (rstd = exp(-0.5*ln(var+eps)) keeps ACT on
    one table set) deferred as fill work into the next chunk's attention.
"""

import numpy as np

import concourse.bass as bass
import concourse.mybir as mybir
import concourse.tile as tile
from concourse import bacc, bass_utils
from concourse.masks import make_identity

B, S, HID, H, HD = 4, 2048, 768, 12, 64
FT = HID // 128   # 6 feature tiles
ST = S // 128     # 16 seq tiles
QT = S // 512     # 4 q chunks
NH = 2            # 768-wide outputs split into 2 x 384
NW = 384
VW = 80           # padded per-head width in packed v (64 ctx + 1 denom + pad)
EPS = 1e-12

F32 = mybir.dt.float32
BF16 = mybir.dt.bfloat16
FP8 = mybir.dt.float8e4
AF = mybir.ActivationFunctionType
DR = mybir.MatmulPerfMode.DoubleRow


def _bcast_part(ap, p=128):
    """DRAM row [1, N] -> partition-broadcast AP [p, N] (stride-0 partition)."""
    return bass.AP(tensor=ap.tensor, offset=ap.offset, ap=[[0, p], ap.ap[-1]])


def build_nc():
    nc = bacc.Bacc("TRN2", target_bir_lowering=False, debug=False, num_devices=8)

    xq_d = nc.dram_tensor("xq", [S, HID], F32, kind="ExternalInput").ap()
    xkv_d = nc.dram_tensor("xkv", [S, HID], F32, kind="ExternalInput").ap()
    wq_d = nc.dram_tensor("wq", [HID, HID], F32, kind="ExternalInput").ap()
    wk_d = nc.dram_tensor("wk", [HID, HID], F32, kind="ExternalInput").ap()
    wv_d = nc.dram_tensor("wv", [HID, HID], F32, kind="ExternalInput").ap()
    wd_d = nc.dram_tensor("wd", [HID, HID], F32, kind="ExternalInput").ap()
    bq_d = nc.dram_tensor("bq", [1, HID], F32, kind="ExternalInput").ap()
    bk_d = nc.dram_tensor("bk", [1, HID], F32, kind="ExternalInput").ap()
    bv_d = nc.dram_tensor("bv", [1, HID], F32, kind="ExternalInput").ap()
    bd_d = nc.dram_tensor("bd", [1, HID], F32, kind="ExternalInput").ap()
    mask_d = nc.dram_tensor("mask", [S, 1], F32, kind="ExternalInput").ap()
    lng_d = nc.dram_tensor("lng", [1, HID], F32, kind="ExternalInput").ap()
    lnb_d = nc.dram_tensor("lnb", [1, HID], F32, kind="ExternalInput").ap()
    out_d = nc.dram_tensor("out", [S, HID], F32, kind="ExternalOutput").ap()

    with tile.TileContext(nc) as tc:
        with (
            tc.tile_pool(name="consts", bufs=1) as consts,
            tc.tile_pool(name="big", bufs=1) as big,
        ):
            # ---- constants ----
            ident = consts.tile([128, 128], F32)
            make_identity(nc, ident)
            ones_r = consts.tile([1, 128], BF16)
            nc.vector.memset(ones_r, 1.0)
            ones_12 = consts.tile([128, 12], F32)
            nc.vector.memset(ones_12, 1.0)
            eps_t = consts.tile([128, 1], F32)
            nc.vector.memset(eps_t, EPS)

            bqc = consts.tile([128, FT], F32)
            bkc = consts.tile([128, FT], F32)
            for f in range(FT):
                nc.sync.dma_start(
                    out=bqc[:, f : f + 1],
                    in_=bq_d[0:1, f * 128 : (f + 1) * 128].rearrange("a b -> b a"),
                )
                nc.sync.dma_start(
                    out=bkc[:, f : f + 1],
                    in_=bk_d[0:1, f * 128 : (f + 1) * 128].rearrange("a b -> b a"),
                )
            bv_f = consts.tile([1, HID], F32)
            nc.sync.dma_start(out=bv_f, in_=bv_d)
            bd_f = consts.tile([1, HID], F32)
            nc.sync.dma_start(out=bd_f, in_=bd_d)
            bv_row = consts.tile([1, HID], BF16)
            nc.vector.tensor_copy(out=bv_row, in_=bv_f)
            bd_row = consts.tile([1, HID], BF16)
            nc.vector.tensor_copy(out=bd_row, in_=bd_f)

            mask_t = consts.tile([128, ST], F32)
            for t in range(ST):
                nc.sync.dma_start(
                    out=mask_t[:, t : t + 1], in_=mask_d[t * 128 : (t + 1) * 128, :]
                )
            emask = consts.tile([128, ST], F32)
            nc.scalar.activation(out=emask, in_=mask_t, func=AF.Exp)

            # broadcast ln gamma/beta to all 128 partitions (stride-0 DMA)
            g_bc = consts.tile([128, HID], F32)
            b_bc = consts.tile([128, HID], F32)
            nc.sync.dma_start(out=g_bc, in_=_bcast_part(lng_d))
            nc.sync.dma_start(out=b_bc, in_=_bcast_part(lnb_d))

            # ---- persistent activation buffers ----
            qT = [big.tile([128, S], BF16, name=f"qT{f}") for f in range(FT)]
            kT = [big.tile([128, S], BF16, name=f"kT{f}") for f in range(FT)]
            # packed v, fp8: [keys, ktile-pair-half, head, 64 ctx + denom]
            vb_dr = [
                big.tile([128, 2, H, VW], FP8, name=f"vbdr{u}") for u in range(8)
            ]
            # wd natural rows, bf16 (dense is K=128 over packed ctx2)
            dw6 = [big.tile([128, HID], BF16, name=f"dw6{f}") for f in range(FT)]
            # transposed x_kv chunks stay alive for the deferred v projection
            xTkv = [
                big.tile([128, FT, 512], BF16, name=f"xTkv{c}") for c in range(QT)
            ]
            wv_b = [big.tile([128, HID], BF16, name=f"wv{f}") for f in range(FT)]
            wq_b = [big.tile([128, HID], BF16, name=f"wq{f}") for f in range(FT)]

            # ---- startup: transpose x_kv, project kT (v deferred) ----
            with (
                tc.tile_pool(name="wk_pool", bufs=1) as wk_pool,
                tc.tile_pool(name="xn2", bufs=3) as xn2_pool,
                tc.tile_pool(name="ps_tp2", bufs=2, space="PSUM") as ps_tp2,
                tc.tile_pool(name="ps_pj2", bufs=2, space="PSUM") as ps_pj2,
            ):
                wk_b = [
                    wk_pool.tile([128, HID], BF16, name=f"wk{f}") for f in range(FT)
                ]
                for f in range(FT):
                    wtmp = xn2_pool.tile([128, HID], F32, name="wtmp2")
                    nc.sync.dma_start(out=wtmp, in_=wk_d[f * 128 : (f + 1) * 128, :])
                    nc.vector.tensor_copy(out=wk_b[f], in_=wtmp)
                    wtmp = xn2_pool.tile([128, HID], F32, name="wtmp2")
                    nc.sync.dma_start(out=wtmp, in_=wv_d[f * 128 : (f + 1) * 128, :])
                    nc.vector.tensor_copy(out=wv_b[f], in_=wtmp)

                for chunk in range(QT):
                    xT_c = xTkv[chunk]
                    for ss in range(4):
                        x_nat = xn2_pool.tile([128, HID], F32, name="x_nat")
                        st = chunk * 4 + ss
                        nc.sync.dma_start(
                            out=x_nat, in_=xkv_d[st * 128 : (st + 1) * 128, :]
                        )
                        for f in range(FT):
                            tp_ps = ps_tp2.tile([128, 128], F32, name="tp_ps")
                            nc.tensor.transpose(
                                tp_ps, x_nat[:, f * 128 : (f + 1) * 128], ident
                            )
                            nc.vector.tensor_copy(
                                out=xT_c[:, f, ss * 128 : (ss + 1) * 128], in_=tp_ps
                            )
                    for fo in range(FT):
                        pj = ps_pj2.tile([128, 512], F32, name="pj2")
                        for kf in range(FT):
                            nc.tensor.matmul(
                                pj,
                                wk_b[kf][:, fo * 128 : (fo + 1) * 128],
                                xT_c[:, kf, :],
                                start=(kf == 0),
                                stop=(kf == FT - 1),
                            )
                        nc.vector.tensor_scalar_add(
                            out=kT[fo][:, chunk * 512 : (chunk + 1) * 512],
                            in0=pj,
                            scalar1=bkc[:, fo : fo + 1],
                        )

            # ---- attention + dense + layernorm ----
            with (
                tc.tile_pool(name="xnq", bufs=3) as xnq_pool,
                tc.tile_pool(name="xTq", bufs=1) as xTq_pool,
                tc.tile_pool(name="ctx2_pool", bufs=2) as ctx2_pool,
                tc.tile_pool(name="exp_pool", bufs=4) as exp_pool,
                tc.tile_pool(name="dram_pool", bufs=2, space="DRAM") as dram_pool,
                tc.tile_pool(name="rec_pool", bufs=2) as rec_pool,
                tc.tile_pool(name="res_pool", bufs=3) as res_pool,
                tc.tile_pool(name="hpre_pool", bufs=1) as hpre_pool,
                tc.tile_pool(name="st_pool", bufs=4) as st_pool,
                tc.tile_pool(name="ps_sc", bufs=2, space="PSUM") as ps_sc,
                tc.tile_pool(name="ps_ctx", bufs=1, space="PSUM") as ps_ctx,
                tc.tile_pool(name="ps_aux", bufs=2, space="PSUM") as ps_aux,
            ):
                for f in range(FT):
                    wtmp = xnq_pool.tile([128, HID], F32, name="x_nat")
                    nc.sync.dma_start(out=wtmp, in_=wq_d[f * 128 : (f + 1) * 128, :])
                    nc.vector.tensor_copy(out=wq_b[f], in_=wtmp)
                for f in range(FT):
                    wtmp = xnq_pool.tile([128, HID], F32, name="x_nat")
                    nc.sync.dma_start(out=wtmp, in_=wd_d[f * 128 : (f + 1) * 128, :])
                    nc.vector.tensor_copy(out=dw6[f], in_=wtmp)

                # --- deferred v projection: one 128-row step, both halves ---
                def v_proj_step(st):
                    u, half = st // 2, st % 2
                    chunk, ss = st // 4, st % 4
                    xT_c = xTkv[chunk]
                    emcol = emask[:, st : st + 1]
                    for nh in range(NH):
                        vp = ps_aux.tile([128, 512], F32, name="aux")
                        for kf in range(FT):
                            nc.tensor.matmul(
                                vp[:, 0:NW],
                                xT_c[:, kf, ss * 128 : (ss + 1) * 128],
                                wv_b[kf][:, nh * NW : (nh + 1) * NW],
                                start=(kf == 0),
                                stop=False,
                            )
                        nc.tensor.matmul(
                            vp[:, 0:NW],
                            ones_r,
                            bv_row[0:1, nh * NW : (nh + 1) * NW],
                            start=False,
                            stop=True,
                        )
                        nc.vector.tensor_scalar_mul(
                            out=vb_dr[u][:, half, nh * 6 : (nh + 1) * 6, 0:HD],
                            in0=vp[:, 0:NW].rearrange("p (a d) -> p a d", a=6),
                            scalar1=emcol,
                        )
                    nc.vector.tensor_scalar_mul(
                        out=vb_dr[u][:, half, :, HD : HD + 1].rearrange(
                            "p a c -> p (a c)"
                        ),
                        in0=ones_12,
                        scalar1=emcol,
                    )

                # --- q projection (fill work) ---
                def q_proj_mm(chunk, xT_c, fo_range):
                    for fo in fo_range:
                        pj = ps_aux.tile([128, 512], F32, name="aux")
                        for kf in range(FT):
                            nc.tensor.matmul(
                                pj,
                                wq_b[kf][:, fo * 128 : (fo + 1) * 128],
                                xT_c[:, kf, :],
                                start=(kf == 0),
                                stop=(kf == FT - 1),
                            )
                        nc.vector.tensor_scalar_add(
                            out=qT[fo][:, chunk * 512 : (chunk + 1) * 512],
                            in0=pj,
                            scalar1=bqc[:, fo : fo + 1],
                        )

                def q_transpose_ss(xT_c, x_nat, ss):
                    for f in range(FT):
                        tp_ps = ps_aux.tile([128, 512], F32, name="aux")
                        nc.tensor.transpose(
                            tp_ps[:, 0:128], x_nat[:, f * 128 : (f + 1) * 128], ident
                        )
                        nc.vector.tensor_copy(
                            out=xT_c[:, f, ss * 128 : (ss + 1) * 128],
                            in_=tp_ps[:, 0:128],
                        )

                def q_proj_steps(chunk):
                    # DMA prefetch is a separate step popped one slot earlier
                    # than the transposes it feeds, so the in-order PE queue
                    # never stalls on DMA latency
                    state = {}

                    def dstep(ss_pair):
                        def run():
                            if "xT" not in state:
                                state["xT"] = xTq_pool.tile(
                                    [128, FT, 512], BF16, name="xT_q"
                                )
                            for ss in ss_pair:
                                x_nat = xnq_pool.tile([128, HID], F32, name="x_nat")
                                st = chunk * 4 + ss
                                nc.sync.dma_start(
                                    out=x_nat,
                                    in_=xq_d[st * 128 : (st + 1) * 128, :],
                                )
                                state[ss] = x_nat

                        return run

                    def tstep(ss_pair):
                        def run():
                            for ss in ss_pair:
                                q_transpose_ss(state["xT"], state[ss], ss)

                        return run

                    def mstep(fo_range):
                        return lambda: q_proj_mm(chunk, state["xT"], fo_range)

                    return [
                        dstep((0, 1)),
                        tstep((0, 1)),
                        dstep((2, 3)),
                        tstep((2, 3)),
                        mstep(range(0, 2)),
                        mstep(range(2, 4)),
                        mstep(range(4, 6)),
                    ]

                # --- dense + residual + LN for chunk qt (fill work) ---
                def make_dense_steps(qt, ctx2t):
                    state = {}

                    def group_step(ss, nh):
                        def run():
                            if "mvq" not in state:
                                state["mvq"] = st_pool.tile(
                                    [128, 4, 2], F32, name="mvq"
                                )
                                state["hp"] = {}
                            st = qt * 4 + ss
                            ssl = slice(ss * 128, (ss + 1) * 128)
                            if ss not in state["hp"]:
                                state["hp"][ss] = hpre_pool.tile(
                                    [128, HID], F32, name=f"hp{ss}"
                                )
                            hp = state["hp"][ss]
                            h_ps = ps_aux.tile([128, 512], F32, name="aux")
                            for f in range(FT):
                                nc.tensor.matmul(
                                    h_ps[:, 0:NW],
                                    ctx2t[:, f, ssl],
                                    dw6[f][:, nh * NW : (nh + 1) * NW],
                                    start=(f == 0),
                                    stop=False,
                                )
                            nc.tensor.matmul(
                                h_ps[:, 0:NW],
                                ones_r,
                                bd_row[0:1, nh * NW : (nh + 1) * NW],
                                start=False,
                                stop=True,
                            )
                            x_res = res_pool.tile([128, NW], F32, name="x_res")
                            nc.sync.dma_start(
                                out=x_res,
                                in_=xkv_d[
                                    st * 128 : (st + 1) * 128,
                                    nh * NW : (nh + 1) * NW,
                                ],
                            )
                            nc.vector.tensor_add(
                                out=hp[:, nh * NW : (nh + 1) * NW],
                                in0=h_ps[:, 0:NW],
                                in1=x_res,
                            )
                            if nh == NH - 1:
                                stats = st_pool.tile([128, 3, 6], F32, name="stats")
                                for sg in range(3):
                                    nc.vector.bn_stats(
                                        out=stats[:, sg, :],
                                        in_=hp[:, sg * 256 : (sg + 1) * 256],
                                    )
                                nc.vector.bn_aggr(
                                    out=state["mvq"][:, ss, :], in_=stats
                                )

                        return run

                    def tail():
                        mvq = state["mvq"]
                        lnv = st_pool.tile([128, 4], F32, name="lnv")
                        nc.scalar.activation(
                            out=lnv, in_=mvq[:, :, 1], func=AF.Ln,
                            bias=eps_t, scale=1.0,
                        )
                        rstd4 = st_pool.tile([128, 4], F32, name="rstd4")
                        nc.scalar.activation(
                            out=rstd4, in_=lnv, func=AF.Exp, scale=-0.5
                        )
                        for ss in range(4):
                            st = qt * 4 + ss
                            hp = state["hp"][ss]
                            hn = hpre_pool.tile([128, HID], F32, name="hn")
                            nc.vector.tensor_scalar(
                                out=hn,
                                in0=hp,
                                scalar1=mvq[:, ss, 0:1],
                                scalar2=rstd4[:, ss : ss + 1],
                                op0=mybir.AluOpType.subtract,
                                op1=mybir.AluOpType.mult,
                            )
                            nc.vector.tensor_mul(hn, hn, g_bc)
                            nc.vector.tensor_add(hn, hn, b_bc)
                            nc.sync.dma_start(
                                out=out_d[st * 128 : (st + 1) * 128, :], in_=hn
                            )

                    return [
                        group_step(ss, nh) for ss in range(4) for nh in range(NH)
                    ] + [tail]

                # chunk 0's qT is needed immediately; emit it directly
                xT0 = xTq_pool.tile([128, FT, 512], BF16, name="xT_q")
                xnat0 = []
                for ss in range(4):
                    x_nat = xnq_pool.tile([128, HID], F32, name="x_nat")
                    nc.sync.dma_start(
                        out=x_nat, in_=xq_d[ss * 128 : (ss + 1) * 128, :]
                    )
                    xnat0.append(x_nat)
                for ss in range(4):
                    q_transpose_ss(xT0, xnat0[ss], ss)
                q_proj_mm(0, xT0, range(FT))

                pending = []

                def pop_fill():
                    if pending:
                        pending.pop(0)()

                def emit_pair(qt, hp, ctx2t, den_all, v_inline, tails, posts):
                    """Attention for head pair (2hp, 2hp+1), q chunk qt.

                    ctx matmuls lag the exps by ~5 steps; the last two ctx
                    groups + the PSUM evict are RETURNED as closures and
                    emitted during the next pair's first steps, so the
                    in-order PE queue never stalls on the final exps at a
                    pair boundary."""
                    qsl = slice(qt * 512, (qt + 1) * 512)
                    ctxA = ps_ctx.tile([HD + 1, 512], F32, name="ctxA")
                    ctxB = ps_ctx.tile([HD + 1, 512], F32, name="ctxB")
                    exps_u = {}

                    def emit_ctx(u):
                        rv = exps_u[u].rearrange(
                            "p (k two) n -> p two k n", two=2
                        )
                        nc.tensor.matmul(
                            ctxA,
                            vb_dr[u][:, :, 2 * hp, 0 : HD + 1],
                            rv[:, 0],
                            start=(u == 0),
                            stop=(u == 7),
                            perf_mode=DR,
                        )
                        nc.tensor.matmul(
                            ctxB,
                            vb_dr[u][:, :, 2 * hp + 1, 0 : HD + 1],
                            rv[:, 1],
                            start=(u == 0),
                            stop=(u == 7),
                            perf_mode=DR,
                        )

                    for kc in range(ST):
                        u, half = kc // 2, kc % 2
                        sc = ps_sc.tile([128, 2, 512], F32, name="sc")
                        nc.tensor.matmul(
                            sc[:, 0, :],
                            kT[hp][0:HD, kc * 128 : (kc + 1) * 128],
                            qT[hp][0:HD, qsl],
                            start=True, stop=True, tile_position=(0, 0),
                        )
                        nc.tensor.matmul(
                            sc[:, 1, :],
                            kT[hp][HD:128, kc * 128 : (kc + 1) * 128],
                            qT[hp][HD:128, qsl],
                            start=True, stop=True, tile_position=(64, 0),
                        )
                        if v_inline:
                            v_proj_step(kc)
                        if half == 0:
                            exps_u[u] = exp_pool.tile(
                                [128, 4, 512], FP8, name="exps"
                            )
                        nc.scalar.activation(
                            out=exps_u[u][:, 2 * half : 2 * half + 2, :],
                            in_=sc, func=AF.Exp, scale=0.125,
                        )
                        if kc <= 1 and tails:
                            tails.pop(0)()
                        elif kc == 2 and posts:
                            posts.pop(0)()
                        if kc >= 5 and half == 1:
                            emit_ctx((kc - 5) // 2)
                            if not v_inline:
                                pop_fill()

                    def evict():
                        # ctx rows into packed ctx2 (partition-shift of 64 is
                        # legal on DVE); denom rows via same-partition copy +
                        # DMA (DVE shifts must be multiples of 32)
                        dtmp = rec_pool.tile([HD + 1, 1024], F32, name="dtmp")
                        nc.vector.tensor_copy(
                            out=ctx2t[0:HD, hp, :], in_=ctxA[0:HD, :]
                        )
                        nc.vector.tensor_copy(
                            out=dtmp[HD : HD + 1, 0:512], in_=ctxA[HD : HD + 1, :]
                        )
                        nc.vector.tensor_copy(
                            out=ctx2t[HD:128, hp, :], in_=ctxB[0:HD, :]
                        )
                        nc.vector.tensor_copy(
                            out=dtmp[HD : HD + 1, 512:1024],
                            in_=ctxB[HD : HD + 1, :],
                        )
                        nc.sync.dma_start(
                            out=den_all[2 * hp : 2 * hp + 2, :],
                            in_=dtmp[HD : HD + 1, :],
                        )

                    def t1():
                        emit_ctx(6)

                    def t2():
                        emit_ctx(7)
                        evict()

                    return [t1, t2]

                def emit_norm(ctx2t, den_all):
                    # batched reciprocal; bf16 partition-broadcast via DRAM
                    # bounce + stride-0 DMA; one in-place 2x multiply per head
                    rec_all = rec_pool.tile([H, 512], F32, name="rec_all")
                    nc.vector.reciprocal_approx_fast(out=rec_all, in_=den_all)
                    rec_bf = rec_pool.tile([H, 512], BF16, name="rec_bf")
                    nc.vector.tensor_copy(out=rec_bf, in_=rec_all)
                    rec_d = dram_pool.tile([H, 512], BF16, name="rec_d")
                    nc.sync.dma_start(out=rec_d, in_=rec_bf)
                    for f in range(FT):
                        # rec rows for heads (2f, 2f+1) stacked on partitions
                        # 0:64 / 64:128 -> one full-width multiply per f-tile
                        bc_sb = rec_pool.tile([128, 512], BF16, name="bc_sb")
                        nc.sync.dma_start(
                            out=bc_sb[0:HD, :],
                            in_=rec_d[2 * f : 2 * f + 1, :].to_broadcast((HD, 512)),
                        )
                        nc.sync.dma_start(
                            out=bc_sb[HD:128, :],
                            in_=rec_d[2 * f + 1 : 2 * f + 2, :].to_broadcast(
                                (HD, 512)
                            ),
                        )
                        nc.vector.tensor_mul(
                            out=ctx2t[:, f, :],
                            in0=ctx2t[:, f, :],
                            in1=bc_sb,
                        )

                tails = []
                posts = []
                for qt in range(QT):
                    if qt + 1 < QT:
                        pending.extend(q_proj_steps(qt + 1))
                    ctx2t = ctx2_pool.tile([128, FT, 512], BF16, name="ctx2")
                    den_all = rec_pool.tile([H, 512], F32, name="den_all")
                    for hp in range(FT):
                        tails = emit_pair(
                            qt, hp, ctx2t, den_all, qt == 0 and hp == 0,
                            tails, posts,
                        )

                    def post(qt=qt, c=ctx2t, d=den_all):
                        emit_norm(c, d)
                        pending.extend(make_dense_steps(qt, c))

                    posts.append(post)
                for t in tails:
                    t()
                for p in posts:
                    p()
                for step in pending:
                    step()

    nc.compile()
    return nc


_NC = None


def _get_nc():
    global _NC
    if _NC is None:
        _NC = build_nc()
    return _NC


def _prepare(
    input_tensor1, attention_mask1, input_tensor2, attention_mask2,
    q1_w, q1_b, k1_w, k1_b, v1_w, v1_b,
    q2_w, q2_b, k2_w, k2_b, v2_w, v2_b,
    d1_w, d1_b, d2_w, d2_b, ln1_g, ln1_b, ln2_g, ln2_b,
):
    f = lambda a: np.ascontiguousarray(np.asarray(a), dtype=np.float32)
    x1, x2 = f(input_tensor1), f(input_tensor2)
    m1 = f(attention_mask1).reshape(B, S, 1)
    m2 = f(attention_mask2).reshape(B, S, 1)
    row = lambda a: f(a).reshape(1, HID)

    in_maps = []
    for b in range(B):
        # stream1: ctx1 = attend(q2, k1, v1, mask1); out h1[b]
        in_maps.append({
            "xq": x2[b], "xkv": x1[b],
            "wq": f(q2_w), "wk": f(k1_w), "wv": f(v1_w), "wd": f(d1_w),
            "bq": row(q2_b), "bk": row(k1_b), "bv": row(v1_b), "bd": row(d1_b),
            "mask": m1[b], "lng": row(ln1_g), "lnb": row(ln1_b),
        })
    for b in range(B):
        # stream2: ctx2 = attend(q1, k2, v2, mask2); out h2[b]
        in_maps.append({
            "xq": x1[b], "xkv": x2[b],
            "wq": f(q1_w), "wk": f(k2_w), "wv": f(v2_w), "wd": f(d2_w),
            "bq": row(q1_b), "bk": row(k2_b), "bv": row(v2_b), "bd": row(d2_b),
            "mask": m2[b], "lng": row(ln2_g), "lnb": row(ln2_b),
        })

    return in_maps


def _run(in_maps, **kwargs):
    nc = _get_nc()
    res = bass_utils.run_bass_kernel_spmd(
        nc, in_maps, core_ids=list(range(8)), **kwargs
    )
    h1 = np.stack([res.results[b]["out"] for b in range(B)])
    h2 = np.stack([res.results[B + b]["out"] for b in range(B)])
    return (h1, h2), res


def kernel(**inputs):
    (h1, h2), _ = _run(_prepare(**inputs))
    return h1, h2


# revision 19
# speedup vs baseline: 1.3052x; 1.1198x over previous
"""BertBiAttention Trainium2 kernel (v2).

Cross-attention between two streams (B=4, S=2048, HID=768, H=12 heads).
Sharding: 8 cores = (stream s in {1,2}) x (batch b in {0..3}). Each core
computes one stream's full output for one batch element:
    h_s[b] = LayerNorm( attend(q_other, k_own, v_own, mask_own) @ wd + bd + x_own )
No collectives needed; the host stacks per-core outputs.

v2 structure (per core):
  - scores: bf16 row-tiled CONCURRENT matmul pairs -- heads (2f, 2f+1) live at
    partition rows 0:64 / 64:128 of qT/kT[f]; tile_position (0,0)/(64,0) runs
    both K=64 matmuls simultaneously on the PE array (~2x score throughput).
  - softmax exp on ACT writes fp8e4 directly; ctx matmuls are fp8 DoubleRow
    (two key-tiles per pass, ~1.8x) with lhsT = packed v [128, 2, 12, 80]
    ([v*exp(mask) | exp(mask)] per head, denom row included, M=65).
  - ctx evicted into ctx2 [128, 6, 512] (heads packed along partitions via
    DVE partition-base-shifted copies) so dense runs K=128 matmuls (2x);
    denominators batched: reciprocal_approx_fast + bf16 DRAM-bounce
    broadcast + one in-place 2x-mode multiply per head.
  - v-projection deferred out of the startup phase: emitted inline between
    pair-0 score steps so it runs while ACT computes exps.
  - dense + residual + LayerNorm (rstd = exp(-0.5*ln(var+eps)) keeps ACT on
    one table set) deferred as fill work into the next chunk's attention.
"""

import numpy as np

import concourse.bass as bass
import concourse.mybir as mybir
import concourse.tile as tile
from concourse import bacc, bass_utils
from concourse.masks import make_identity

B, S, HID, H, HD = 4, 2048, 768, 12, 64
FT = HID // 128   # 6 feature tiles
ST = S // 128     # 16 seq tiles
QT = S // 512     # 4 q chunks
NH = 2            # 768-wide outputs split into 2 x 384
NW = 384
VW = 80           # padded per-head width in packed v (64 ctx + 1 denom + pad)
EPS = 1e-12

F32 = mybir.dt.float32
BF16 = mybir.dt.bfloat16
FP8 = mybir.dt.float8e4
AF = mybir.ActivationFunctionType
DR = mybir.MatmulPerfMode.DoubleRow


def _bcast_part(ap, p=128):
    """DRAM row [1, N] -> partition-broadcast AP [p, N] (stride-0 partition)."""
    return bass.AP(tensor=ap.tensor, offset=ap.offset, ap=[[0, p], ap.ap[-1]])


def build_nc():
    nc = bacc.Bacc("TRN2", target_bir_lowering=False, debug=False, num_devices=8)

    xq_d = nc.dram_tensor("xq", [S, HID], F32, kind="ExternalInput").ap()
    xkv_d = nc.dram_tensor("xkv", [S, HID], F32, kind="ExternalInput").ap()
    wq_d = nc.dram_tensor("wq", [HID, HID], F32, kind="ExternalInput").ap()
    wk_d = nc.dram_tensor("wk", [HID, HID], F32, kind="ExternalInput").ap()
    wv_d = nc.dram_tensor("wv", [HID, HID], F32, kind="ExternalInput").ap()
    wd_d = nc.dram_tensor("wd", [HID, HID], F32, kind="ExternalInput").ap()
    bq_d = nc.dram_tensor("bq", [1, HID], F32, kind="ExternalInput").ap()
    bk_d = nc.dram_tensor("bk", [1, HID], F32, kind="ExternalInput").ap()
    bv_d = nc.dram_tensor("bv", [1, HID], F32, kind="ExternalInput").ap()
    bd_d = nc.dram_tensor("bd", [1, HID], F32, kind="ExternalInput").ap()
    mask_d = nc.dram_tensor("mask", [S, 1], F32, kind="ExternalInput").ap()
    lng_d = nc.dram_tensor("lng", [1, HID], F32, kind="ExternalInput").ap()
    lnb_d = nc.dram_tensor("lnb", [1, HID], F32, kind="ExternalInput").ap()
    out_d = nc.dram_tensor("out", [S, HID], F32, kind="ExternalOutput").ap()

    with tile.TileContext(nc) as tc:
        with (
            tc.tile_pool(name="consts", bufs=1) as consts,
            tc.tile_pool(name="big", bufs=1) as big,
        ):
            # ---- constants ----
            ident = consts.tile([128, 128], F32)
            make_identity(nc, ident)
            ones_r = consts.tile([1, 128], BF16)
            nc.vector.memset(ones_r, 1.0)
            ones_12 = consts.tile([128, 12], F32)
            nc.vector.memset(ones_12, 1.0)
            eps_t = consts.tile([128, 1], F32)
            nc.vector.memset(eps_t, EPS)

            bqc = consts.tile([128, FT], F32)
            bkc = consts.tile([128, FT], F32)
            for f in range(FT):
                nc.sync.dma_start(
                    out=bqc[:, f : f + 1],
                    in_=bq_d[0:1, f * 128 : (f + 1) * 128].rearrange("a b -> b a"),
                )
                nc.sync.dma_start(
                    out=bkc[:, f : f + 1],
                    in_=bk_d[0:1, f * 128 : (f + 1) * 128].rearrange("a b -> b a"),
                )
            bv_f = consts.tile([1, HID], F32)
            nc.sync.dma_start(out=bv_f, in_=bv_d)
            bd_f = consts.tile([1, HID], F32)
            nc.sync.dma_start(out=bd_f, in_=bd_d)
            # v projection runs in fp8 with weights scaled by 16; bias is
            # added inside the PSUM accumulation, so pre-scale it too
            bv_row = consts.tile([1, HID], BF16)
            nc.vector.tensor_scalar_mul(out=bv_row, in0=bv_f, scalar1=16.0)
            bd_row = consts.tile([1, HID], BF16)
            nc.vector.tensor_copy(out=bd_row, in_=bd_f)

            mask_t = consts.tile([128, ST], F32)
            for t in range(ST):
                nc.sync.dma_start(
                    out=mask_t[:, t : t + 1], in_=mask_d[t * 128 : (t + 1) * 128, :]
                )
            emask = consts.tile([128, ST], F32)
            nc.scalar.activation(out=emask, in_=mask_t, func=AF.Exp)
            emask16 = consts.tile([128, ST], F32)
            nc.vector.tensor_scalar_mul(out=emask16, in0=emask, scalar1=1.0 / 16.0)

            # broadcast ln gamma/beta to all 128 partitions (stride-0 DMA)
            g_bc = consts.tile([128, HID], F32)
            b_bc = consts.tile([128, HID], F32)
            nc.sync.dma_start(out=g_bc, in_=_bcast_part(lng_d))
            nc.sync.dma_start(out=b_bc, in_=_bcast_part(lnb_d))

            # ---- persistent activation buffers ----
            qT = [big.tile([128, S], BF16, name=f"qT{f}") for f in range(FT)]
            kT = [big.tile([128, S], BF16, name=f"kT{f}") for f in range(FT)]
            # packed v, fp8: [keys, ktile-pair-half, head, 64 ctx + denom]
            vb_dr = [
                big.tile([128, 2, H, VW], FP8, name=f"vbdr{u}") for u in range(8)
            ]
            # wd natural rows, bf16 (dense is K=128 over packed ctx2)
            dw6 = [big.tile([128, HID], BF16, name=f"dw6{f}") for f in range(FT)]
            # transposed x_kv chunks stay alive for the deferred v projection
            xTkv = [
                big.tile([128, FT, 512], FP8, name=f"xTkv{c}") for c in range(QT)
            ]
            # q/k/v projection weights: fp8, scaled by 16 (keeps the ~N(0,
            # 0.02) values out of e4m3's subnormal range), packed as
            # [hid_in, 2 k-subtiles, hid_out] for DoubleRow matmuls
            wv_p = [big.tile([128, 2, HID], FP8, name=f"wvp{j}") for j in range(3)]
            wq_p = [big.tile([128, 2, HID], FP8, name=f"wqp{j}") for j in range(3)]
            wk_p = [big.tile([128, 2, HID], FP8, name=f"wkp{j}") for j in range(3)]

            # ---- startup: transpose x_kv, project kT (v deferred) ----
            with (
                tc.tile_pool(name="xn2", bufs=3) as xn2_pool,
                tc.tile_pool(name="ps_tp2", bufs=2, space="PSUM") as ps_tp2,
                tc.tile_pool(name="ps_pj2", bufs=2, space="PSUM") as ps_pj2,
            ):
                def wpack(dst, src_d):
                    for j in range(3):
                        for i in range(2):
                            f = 2 * j + i
                            wtmp = xn2_pool.tile([128, HID], F32, name="wtmp2")
                            nc.sync.dma_start(
                                out=wtmp, in_=src_d[f * 128 : (f + 1) * 128, :]
                            )
                            nc.vector.tensor_scalar_mul(
                                out=dst[j][:, i, :], in0=wtmp, scalar1=16.0
                            )

                wpack(wk_p, wk_d)

                for chunk in range(QT):
                    xT_c = xTkv[chunk]
                    for ss in range(4):
                        x_nat = xn2_pool.tile([128, HID], F32, name="x_nat")
                        st = chunk * 4 + ss
                        nc.sync.dma_start(
                            out=x_nat, in_=xkv_d[st * 128 : (st + 1) * 128, :]
                        )
                        # 6 transposes into one PSUM tile, ONE batched DVE
                        # eviction (the per-tile copies were the startup
                        # critical path: 6x290ns -> 1x960ns)
                        tp6 = ps_tp2.tile([128, FT, 128], F32, name="tp_ps")
                        for f in range(FT):
                            nc.tensor.transpose(
                                tp6[:, f, :], x_nat[:, f * 128 : (f + 1) * 128],
                                ident,
                            )
                        nc.vector.tensor_copy(
                            out=xT_c[:, :, ss * 128 : (ss + 1) * 128], in_=tp6
                        )
                    # only kT[0] inline (pair 0 needs it immediately);
                    # fo 1..5 are deferred as attention-phase fill work
                    pj = ps_pj2.tile([128, 512], F32, name="pj2")
                    for j in range(3):
                        nc.tensor.matmul(
                            pj,
                            wk_p[j][:, :, 0:128],
                            xT_c[:, 2 * j : 2 * j + 2, :],
                            start=(j == 0),
                            stop=(j == 2),
                            perf_mode=DR,
                        )
                    nc.vector.tensor_scalar(
                        out=kT[0][:, chunk * 512 : (chunk + 1) * 512],
                        in0=pj,
                        scalar1=1.0 / 16.0,
                        scalar2=bkc[:, 0:1],
                        op0=mybir.AluOpType.mult,
                        op1=mybir.AluOpType.add,
                    )

                # wv casts last: v projection first consumes them ~17us in
                wpack(wv_p, wv_d)

            # ---- attention + dense + layernorm ----
            with (
                tc.tile_pool(name="xnq", bufs=3) as xnq_pool,
                tc.tile_pool(name="xTq", bufs=2) as xTq_pool,
                tc.tile_pool(name="ctx2_pool", bufs=2) as ctx2_pool,
                tc.tile_pool(name="exp_pool", bufs=4) as exp_pool,
                tc.tile_pool(name="dram_pool", bufs=2, space="DRAM") as dram_pool,
                tc.tile_pool(name="rec_pool", bufs=2) as rec_pool,
                tc.tile_pool(name="res_pool", bufs=2) as res_pool,
                tc.tile_pool(name="dt_pool", bufs=1) as dt_pool,
                tc.tile_pool(name="hpre_pool", bufs=1) as hpre_pool,
                tc.tile_pool(name="st_pool", bufs=2) as st_pool,
                tc.tile_pool(name="ps_sc", bufs=2, space="PSUM") as ps_sc,
                tc.tile_pool(name="ps_ctx", bufs=1, space="PSUM") as ps_ctx,
                tc.tile_pool(name="ps_aux", bufs=2, space="PSUM") as ps_aux,
            ):
                def wload_fill(dst, src_d, f3):
                    def run():
                        for f in f3:
                            wtmp = xnq_pool.tile([128, HID], F32, name="x_nat")
                            nc.sync.dma_start(
                                out=wtmp, in_=src_d[f * 128 : (f + 1) * 128, :]
                            )
                            nc.vector.tensor_copy(out=dst[f], in_=wtmp)

                    return run

                def kt_fill(fo, chunk):
                    def run():
                        pj = ps_aux.tile([128, 512], F32, name="aux")
                        for j in range(3):
                            nc.tensor.matmul(
                                pj,
                                wk_p[j][:, :, fo * 128 : (fo + 1) * 128],
                                xTkv[chunk][:, 2 * j : 2 * j + 2, :],
                                start=(j == 0),
                                stop=(j == 2),
                                perf_mode=DR,
                            )
                        nc.vector.tensor_scalar(
                            out=kT[fo][:, chunk * 512 : (chunk + 1) * 512],
                            in0=pj,
                            scalar1=1.0 / 16.0,
                            scalar2=bkc[:, fo : fo + 1],
                            op0=mybir.AluOpType.mult,
                            op1=mybir.AluOpType.add,
                        )

                    return run

                # --- deferred v projection: one 128-row step, both halves ---
                def v_proj_step(st):
                    u, half = st // 2, st % 2
                    chunk, ss = st // 4, st % 4
                    xT_c = xTkv[chunk]
                    for nh in range(NH):
                        vp = ps_aux.tile([128, 512], F32, name="aux")
                        for j in range(3):
                            nc.tensor.matmul(
                                vp[:, 0:NW],
                                xT_c[:, 2 * j : 2 * j + 2, ss * 128 : (ss + 1) * 128],
                                wv_p[j][:, :, nh * NW : (nh + 1) * NW],
                                start=(j == 0),
                                stop=False,
                                perf_mode=DR,
                            )
                        nc.tensor.matmul(
                            vp[:, 0:NW],
                            ones_r,
                            bv_row[0:1, nh * NW : (nh + 1) * NW],
                            start=False,
                            stop=True,
                        )
                        nc.vector.tensor_scalar_mul(
                            out=vb_dr[u][:, half, nh * 6 : (nh + 1) * 6, 0:HD],
                            in0=vp[:, 0:NW].rearrange("p (a d) -> p a d", a=6),
                            scalar1=emask16[:, st : st + 1],
                        )
                    nc.vector.tensor_scalar_mul(
                        out=vb_dr[u][:, half, :, HD : HD + 1].rearrange(
                            "p a c -> p (a c)"
                        ),
                        in0=ones_12,
                        scalar1=emask[:, st : st + 1],
                    )

                # --- q projection (fill work) ---
                def q_proj_mm(chunk, xT_c, fo_range):
                    for fo in fo_range:
                        pj = ps_aux.tile([128, 512], F32, name="aux")
                        for j in range(3):
                            nc.tensor.matmul(
                                pj,
                                wq_p[j][:, :, fo * 128 : (fo + 1) * 128],
                                xT_c[:, 2 * j : 2 * j + 2, :],
                                start=(j == 0),
                                stop=(j == 2),
                                perf_mode=DR,
                            )
                        nc.vector.tensor_scalar(
                            out=qT[fo][:, chunk * 512 : (chunk + 1) * 512],
                            in0=pj,
                            scalar1=1.0 / 16.0,
                            scalar2=bqc[:, fo : fo + 1],
                            op0=mybir.AluOpType.mult,
                            op1=mybir.AluOpType.add,
                        )

                def q_transpose_ss(xT_c, x_nat, ss):
                    for f0, nf in ((0, 4), (4, 2)):
                        tp_ps = ps_aux.tile([128, 512], F32, name="aux")
                        tpv = tp_ps.rearrange("p (a b) -> p a b", b=128)
                        for j in range(nf):
                            nc.tensor.transpose(
                                tpv[:, j, :],
                                x_nat[:, (f0 + j) * 128 : (f0 + j + 1) * 128],
                                ident,
                            )
                        nc.vector.tensor_copy(
                            out=xT_c[:, f0 : f0 + nf, ss * 128 : (ss + 1) * 128],
                            in_=tpv[:, 0:nf, :],
                        )

                def q_proj_steps(chunk):
                    # DMA prefetch is a separate step popped one slot earlier
                    # than the transposes it feeds, so the in-order PE queue
                    # never stalls on DMA latency
                    state = {}

                    def dstep(ss_pair):
                        def run():
                            if "xT" not in state:
                                state["xT"] = xTq_pool.tile(
                                    [128, FT, 512], FP8, name="xT_q"
                                )
                            for ss in ss_pair:
                                x_nat = xnq_pool.tile([128, HID], F32, name="x_nat")
                                st = chunk * 4 + ss
                                nc.sync.dma_start(
                                    out=x_nat,
                                    in_=xq_d[st * 128 : (st + 1) * 128, :],
                                )
                                state[ss] = x_nat

                        return run

                    def tstep(ss_pair):
                        def run():
                            for ss in ss_pair:
                                q_transpose_ss(state["xT"], state[ss], ss)

                        return run

                    def mstep(fo_range):
                        return lambda: q_proj_mm(chunk, state["xT"], fo_range)

                    return [
                        dstep((0, 1)),
                        tstep((0, 1)),
                        dstep((2, 3)),
                        tstep((2, 3)),
                        mstep(range(0, 2)),
                        mstep(range(2, 4)),
                        mstep(range(4, 6)),
                    ]

                # --- dense + residual + LN for chunk qt (fill work) ---
                def make_dense_steps(qt, ctx2t):
                    state = {}

                    def group_step(ss, nh):
                        def run():
                            if "mvq" not in state:
                                state["mvq"] = st_pool.tile(
                                    [128, 4, 2], F32, name="mvq"
                                )
                                state["hp"] = {}
                            st = qt * 4 + ss
                            ssl = slice(ss * 128, (ss + 1) * 128)
                            if ss not in state["hp"]:
                                state["hp"][ss] = hpre_pool.tile(
                                    [128, HID], F32, name=f"hp{ss}"
                                )
                            hp = state["hp"][ss]
                            h_ps = ps_aux.tile([128, 512], F32, name="aux")
                            for f in range(FT):
                                nc.tensor.matmul(
                                    h_ps[:, 0:NW],
                                    ctx2t[:, f, ssl],
                                    dw6[f][:, nh * NW : (nh + 1) * NW],
                                    start=(f == 0),
                                    stop=False,
                                )
                            nc.tensor.matmul(
                                h_ps[:, 0:NW],
                                ones_r,
                                bd_row[0:1, nh * NW : (nh + 1) * NW],
                                start=False,
                                stop=True,
                            )
                            x_res = res_pool.tile([128, NW], F32, name="x_res")
                            nc.sync.dma_start(
                                out=x_res,
                                in_=xkv_d[
                                    st * 128 : (st + 1) * 128,
                                    nh * NW : (nh + 1) * NW,
                                ],
                            )
                            nc.vector.tensor_add(
                                out=hp[:, nh * NW : (nh + 1) * NW],
                                in0=h_ps[:, 0:NW],
                                in1=x_res,
                            )
                            if nh == NH - 1:
                                stats = st_pool.tile([128, 3, 6], F32, name="stats")
                                for sg in range(3):
                                    nc.vector.bn_stats(
                                        out=stats[:, sg, :],
                                        in_=hp[:, sg * 256 : (sg + 1) * 256],
                                    )
                                nc.vector.bn_aggr(
                                    out=state["mvq"][:, ss, :], in_=stats
                                )

                        return run

                    def tail():
                        mvq = state["mvq"]
                        lnv = st_pool.tile([128, 4], F32, name="lnv")
                        nc.scalar.activation(
                            out=lnv, in_=mvq[:, :, 1], func=AF.Ln,
                            bias=eps_t, scale=1.0,
                        )
                        rstd4 = st_pool.tile([128, 4], F32, name="rstd4")
                        nc.scalar.activation(
                            out=rstd4, in_=lnv, func=AF.Exp, scale=-0.5
                        )
                        for ss in range(4):
                            st = qt * 4 + ss
                            hp = state["hp"][ss]
                            nc.vector.tensor_scalar(
                                out=hp,
                                in0=hp,
                                scalar1=mvq[:, ss, 0:1],
                                scalar2=rstd4[:, ss : ss + 1],
                                op0=mybir.AluOpType.subtract,
                                op1=mybir.AluOpType.mult,
                            )
                            nc.vector.tensor_mul(hp, hp, g_bc)
                            nc.vector.tensor_add(hp, hp, b_bc)
                            nc.sync.dma_start(
                                out=out_d[st * 128 : (st + 1) * 128, :], in_=hp
                            )

                    return [
                        group_step(ss, nh) for ss in range(4) for nh in range(NH)
                    ] + [tail]

                # wq is read by the inline chunk-0 q projection
                for j in range(3):
                    for i in range(2):
                        f = 2 * j + i
                        wtmp = xnq_pool.tile([128, HID], F32, name="x_nat")
                        nc.sync.dma_start(
                            out=wtmp, in_=wq_d[f * 128 : (f + 1) * 128, :]
                        )
                        nc.vector.tensor_scalar_mul(
                            out=wq_p[j][:, i, :], in0=wtmp, scalar1=16.0
                        )
                # chunk 0's qT is needed immediately; emit it directly
                xT0 = xTq_pool.tile([128, FT, 512], FP8, name="xT_q")
                xnat0 = []
                for ss in range(4):
                    x_nat = xnq_pool.tile([128, HID], F32, name="x_nat")
                    nc.sync.dma_start(
                        out=x_nat, in_=xq_d[ss * 128 : (ss + 1) * 128, :]
                    )
                    xnat0.append(x_nat)
                for ss in range(4):
                    q_transpose_ss(xT0, xnat0[ss], ss)
                q_proj_mm(0, xT0, range(1))

                pending = []
                pending.append(wload_fill(dw6, wd_d, range(0, 3)))
                pending.append(wload_fill(dw6, wd_d, range(3, 6)))

                def pop_fill():
                    if pending:
                        pending.pop(0)()

                def emit_pair(qt, hp, ctx2t, den_all, v_inline, tails, posts,
                              pre):
                    """Attention for head pair (2hp, 2hp+1), q chunk qt.

                    ctx matmuls lag the exps by ~5 steps; the last two ctx
                    groups + the PSUM evict are RETURNED as closures and
                    emitted during the next pair's first steps, so the
                    in-order PE queue never stalls on the final exps at a
                    pair boundary."""
                    qsl = slice(qt * 512, (qt + 1) * 512)
                    ctxA = ps_ctx.tile([HD + 1, 512], F32, name="ctxA")
                    ctxB = ps_ctx.tile([HD + 1, 512], F32, name="ctxB")
                    exps_u = {}

                    def emit_ctx(u):
                        rv = exps_u[u].rearrange(
                            "p (k two) n -> p two k n", two=2
                        )
                        nc.tensor.matmul(
                            ctxA,
                            vb_dr[u][:, :, 2 * hp, 0 : HD + 1],
                            rv[:, 0],
                            start=(u == 0),
                            stop=(u == 7),
                            perf_mode=DR,
                        )
                        nc.tensor.matmul(
                            ctxB,
                            vb_dr[u][:, :, 2 * hp + 1, 0 : HD + 1],
                            rv[:, 1],
                            start=(u == 0),
                            stop=(u == 7),
                            perf_mode=DR,
                        )

                    for kc in range(ST):
                        u, half = kc // 2, kc % 2
                        sc = ps_sc.tile([128, 2, 512], F32, name="sc")
                        nc.tensor.matmul(
                            sc[:, 0, :],
                            kT[hp][0:HD, kc * 128 : (kc + 1) * 128],
                            qT[hp][0:HD, qsl],
                            start=True, stop=True, tile_position=(0, 0),
                        )
                        nc.tensor.matmul(
                            sc[:, 1, :],
                            kT[hp][HD:128, kc * 128 : (kc + 1) * 128],
                            qT[hp][HD:128, qsl],
                            start=True, stop=True, tile_position=(64, 0),
                        )
                        if v_inline:
                            v_proj_step(kc)
                        if half == 0:
                            exps_u[u] = exp_pool.tile(
                                [128, 4, 512], FP8, name="exps"
                            )
                        nc.scalar.activation(
                            out=exps_u[u][:, 2 * half : 2 * half + 2, :],
                            in_=sc, func=AF.Exp, scale=0.125,
                        )
                        if kc <= 1 and tails:
                            tails.pop(0)()
                        elif kc == 2 and posts:
                            posts.pop(0)()
                        elif kc <= 4 and pre:
                            pre.pop(0)()
                        if kc >= 5 and half == 1:
                            emit_ctx((kc - 5) // 2)
                            if pre:
                                pre.pop(0)()
                            else:
                                pop_fill()

                    def evict():
                        # ctx rows into packed ctx2 (partition-shift of 64 is
                        # legal on DVE); denom rows via same-partition copy +
                        # DMA (DVE shifts must be multiples of 32)
                        dtmp = dt_pool.tile([HD + 1, 1024], F32, name="dtmp")
                        nc.vector.tensor_copy(
                            out=ctx2t[0:HD, hp, :], in_=ctxA[0:HD, :]
                        )
                        nc.vector.tensor_copy(
                            out=dtmp[HD : HD + 1, 0:512], in_=ctxA[HD : HD + 1, :]
                        )
                        nc.vector.tensor_copy(
                            out=ctx2t[HD:128, hp, :], in_=ctxB[0:HD, :]
                        )
                        nc.vector.tensor_copy(
                            out=dtmp[HD : HD + 1, 512:1024],
                            in_=ctxB[HD : HD + 1, :],
                        )
                        nc.sync.dma_start(
                            out=den_all[2 * hp : 2 * hp + 2, :],
                            in_=dtmp[HD : HD + 1, :],
                        )

                    def t1():
                        emit_ctx(6)

                    def t2():
                        emit_ctx(7)
                        evict()

                    return [t1, t2]

                def emit_norm(ctx2t, den_all):
                    # batched reciprocal; bf16 partition-broadcast via DRAM
                    # bounce + stride-0 DMA; one in-place 2x multiply per head
                    rec_all = rec_pool.tile([H, 512], F32, name="rec_all")
                    nc.vector.reciprocal_approx_fast(out=rec_all, in_=den_all)
                    rec_bf = rec_pool.tile([H, 512], BF16, name="rec_bf")
                    nc.vector.tensor_copy(out=rec_bf, in_=rec_all)
                    rec_d = dram_pool.tile([H, 512], BF16, name="rec_d")
                    nc.sync.dma_start(out=rec_d, in_=rec_bf)
                    for f in range(FT):
                        # rec rows for heads (2f, 2f+1) stacked on partitions
                        # 0:64 / 64:128 -> one full-width multiply per f-tile
                        bc_sb = rec_pool.tile([128, 512], BF16, name="bc_sb")
                        nc.sync.dma_start(
                            out=bc_sb[0:HD, :],
                            in_=rec_d[2 * f : 2 * f + 1, :].to_broadcast((HD, 512)),
                        )
                        nc.sync.dma_start(
                            out=bc_sb[HD:128, :],
                            in_=rec_d[2 * f + 1 : 2 * f + 2, :].to_broadcast(
                                (HD, 512)
                            ),
                        )
                        nc.vector.tensor_mul(
                            out=ctx2t[:, f, :],
                            in0=ctx2t[:, f, :],
                            in1=bc_sb,
                        )

                tails = []
                posts = []
                for qt in range(QT):
                    if qt + 1 < QT:
                        pending.extend(q_proj_steps(qt + 1))
                    ctx2t = ctx2_pool.tile([128, FT, 512], BF16, name="ctx2")
                    den_all = rec_pool.tile([H, 512], F32, name="den_all")
                    for hp in range(FT):
                        # chunk-0 deferred projections: pair hp emits the
                        # prerequisites of pair hp+1 (its qT f-tile + kT) in
                        # its early steps -- deterministic, never races
                        if qt == 0 and hp + 1 < FT:
                            pre = [
                                lambda fo=hp + 1: q_proj_mm(
                                    0, xT0, range(fo, fo + 1)
                                )
                            ] + [kt_fill(hp + 1, c) for c in range(QT)]
                        else:
                            pre = []
                        tails = emit_pair(
                            qt, hp, ctx2t, den_all, qt == 0 and hp == 0,
                            tails, posts, pre,
                        )

                    def post(qt=qt, c=ctx2t, d=den_all):
                        emit_norm(c, d)
                        pending.extend(make_dense_steps(qt, c))

                    posts.append(post)
                for t in tails:
                    t()
                for p in posts:
                    p()
                for step in pending:
                    step()

    nc.compile()
    return nc


_NC = None


def _get_nc():
    global _NC
    if _NC is None:
        _NC = build_nc()
    return _NC


def _prepare(
    input_tensor1, attention_mask1, input_tensor2, attention_mask2,
    q1_w, q1_b, k1_w, k1_b, v1_w, v1_b,
    q2_w, q2_b, k2_w, k2_b, v2_w, v2_b,
    d1_w, d1_b, d2_w, d2_b, ln1_g, ln1_b, ln2_g, ln2_b,
):
    f = lambda a: np.ascontiguousarray(np.asarray(a), dtype=np.float32)
    x1, x2 = f(input_tensor1), f(input_tensor2)
    m1 = f(attention_mask1).reshape(B, S, 1)
    m2 = f(attention_mask2).reshape(B, S, 1)
    row = lambda a: f(a).reshape(1, HID)

    in_maps = []
    for b in range(B):
        # stream1: ctx1 = attend(q2, k1, v1, mask1); out h1[b]
        in_maps.append({
            "xq": x2[b], "xkv": x1[b],
            "wq": f(q2_w), "wk": f(k1_w), "wv": f(v1_w), "wd": f(d1_w),
            "bq": row(q2_b), "bk": row(k1_b), "bv": row(v1_b), "bd": row(d1_b),
            "mask": m1[b], "lng": row(ln1_g), "lnb": row(ln1_b),
        })
    for b in range(B):
        # stream2: ctx2 = attend(q1, k2, v2, mask2); out h2[b]
        in_maps.append({
            "xq": x1[b], "xkv": x2[b],
            "wq": f(q1_w), "wk": f(k2_w), "wv": f(v2_w), "wd": f(d2_w),
            "bq": row(q1_b), "bk": row(k2_b), "bv": row(v2_b), "bd": row(d2_b),
            "mask": m2[b], "lng": row(ln2_g), "lnb": row(ln2_b),
        })

    return in_maps


def _run(in_maps, **kwargs):
    nc = _get_nc()
    res = bass_utils.run_bass_kernel_spmd(
        nc, in_maps, core_ids=list(range(8)), **kwargs
    )
    h1 = np.stack([res.results[b]["out"] for b in range(B)])
    h2 = np.stack([res.results[B + b]["out"] for b in range(B)])
    return (h1, h2), res


def kernel(**inputs):
    (h1, h2), _ = _run(_prepare(**inputs))
    return h1, h2
